# revision 5
# baseline (speedup 1.0000x reference)
"""Distributed GQA attention block for TRN2 (8 NeuronCores).

Sharding: core = b*4 + g  (b = batch 0..1, g = kv-head-pair 0..3).
Each core computes qkv for its 8 q-heads / 2 kv-heads, full attention for
those heads, and a partial c_proj ([2048,4096]); host sums the 4 partials
per batch and adds c_proj bias.

All PE-facing tensors are bf16 (fp32 PSUM accumulation). Softmax runs
without max-subtraction (scores are O(30), safe in fp32 exp), and the
additive mask is applied as exp(s+m) = exp(s)*exp(m) with exp(m)
precomputed on host, so ScalarE exps raw PSUM scores directly.

The mask is classified per 128x128 block (skip / identity / general), so
for a causal mask the score+exp+AV work shrinks to the lower-triangular
blocks and the exp(m) multiply runs only on diagonal-crossing blocks.
"""
import sys, os, types

sys.path.insert(0, '/opt/trn_rl_repo')

# Inject the NTFF profile hook module that this image's antenv lacks
# (needed only when tracing; harmless otherwise).
try:
    import antenv
    if "antenv.axon_hooks" not in sys.modules:
        _m = types.ModuleType("antenv.axon_hooks")
        _m._hook = None
        def _set(h, _m=_m): _m._hook = h
        def _get(_m=_m): return _m._hook
        _m.set_axon_ntff_profile_hook = _set
        _m.get_axon_ntff_profile_hook = _get
        sys.modules["antenv.axon_hooks"] = _m
        antenv.axon_hooks = _m
        try:
            from trn_agent_boot.trn_boot import _ntff_profile_via_ctypes
            _set(_ntff_profile_via_ctypes('/opt/axon/libaxon_pjrt.so'))
        except Exception:
            pass
except Exception:
    pass

import numpy as np
import ml_dtypes

import concourse.bass as bass
import concourse.tile as tile
from concourse import bacc, mybir
from concourse.bass_utils import run_bass_kernel_spmd

BF16 = mybir.dt.bfloat16
F32 = mybir.dt.float32
BNP = ml_dtypes.bfloat16

B, S, H = 2, 2048, 4096
NH, NKV, HD = 32, 8, 128
G = NH // NKV                  # 4 q heads per kv head
QH = 8                         # q heads per core
KVH = 2                        # kv heads per core
FT = QH + KVH                  # 10 qk feature tiles per core
ST = S // 128                  # 16 s tiles
KBL = H // 128                 # 32 contraction blocks
SCQ = 256                      # qkv-phase seq chunk
NCQ = S // SCQ                 # 8
QC = 512                       # attention qs chunk
NQC = S // QC                  # 4
NT = QC // 128                 # 4 qs subtiles per chunk
ST2 = ST // 2                  # paired score-tile groups (1024-wide psum)
PC = 512                       # c_proj n chunk
NPC = H // PC                  # 8
SCALE = 1.0 / float(np.sqrt(HD))
VW = HD + 1                    # v-aug row width (ones column for softmax Z)

_CACHE = {}
LAST_EXEC_NS = None
LAST_RESULTS = None


def _build_nc(pattern):
    # pattern[qc][kt][j] classifies the [128 ks x 128 qs] block
    # (ks tile kt, qs subtile j of chunk qc) of exp(mask):
    #   0 => identically zero: block skipped entirely (exact).
    #   1 => identically one: computed, no mask multiply.
    #   2 => general: computed, multiplied by exp(mask).
    nc = bacc.Bacc("TRN2", target_bir_lowering=False, debug=False, num_devices=8)

    xt_e = nc.declare_dram_parameter("xt", [NCQ, 128, KBL * SCQ], BF16, isOutput=False)
    wqk_e = nc.declare_dram_parameter("wqk", [FT, 128, KBL * 128], BF16, isOutput=False)
    wv_e = nc.declare_dram_parameter("wv", [128, KBL * KVH * HD], BF16, isOutput=False)
    bqk_e = nc.declare_dram_parameter("bqk", [128, FT], F32, isOutput=False)
    bv_e = nc.declare_dram_parameter("bv", [128, KVH * HD], F32, isOutput=False)
    cos_e = nc.declare_dram_parameter("cos", [128, S], BF16, isOutput=False)
    sins_e = nc.declare_dram_parameter("sins", [128, S], BF16, isOutput=False)
    em_e = nc.declare_dram_parameter("emask", [NQC, ST2, 128, 2 * QC], BF16,
                                     isOutput=False)
    wp_e = nc.declare_dram_parameter("wp", [NPC, 128, QH * PC], BF16, isOutput=False)
    id_e = nc.declare_dram_parameter("ident", [128, 128], BF16, isOutput=False)
    out_e = nc.declare_dram_parameter("out", [S, H], F32, isOutput=True)

    ADD = mybir.AluOpType.add
    MUL = mybir.AluOpType.mult
    EXP = mybir.ActivationFunctionType.Exp

    # ---- pattern-derived helpers (all build-time constants) ----
    def blk(qc, kt, j):
        return pattern[qc][kt][j]

    def jmin(qc, kt):
        act = [j for j in range(NT) if blk(qc, kt, j)]
        return act[0] if act else None

    def mrange(qc, kt):
        # qs-subtile range needing the exp(mask) multiply: class-2 blocks,
        # plus class-0 blocks sitting inside the computed range (their
        # exp(mask)=0 zeroes the computed scores).
        jm = jmin(qc, kt)
        if jm is None:
            return None
        need = [j for j in range(jm, NT) if blk(qc, kt, j) != 1]
        if not need:
            return None
        return (need[0], need[-1])

    def act_pairs(qc):
        return [k2 for k2 in range(ST2)
                if jmin(qc, 2 * k2) is not None or jmin(qc, 2 * k2 + 1) is not None]

    def pair_needs_mask(qc, k2):
        return (mrange(qc, 2 * k2) is not None or
                mrange(qc, 2 * k2 + 1) is not None)

    with tile.TileContext(nc) as tc:
        from contextlib import ExitStack
        with ExitStack() as ctx:
            persist = ctx.enter_context(tc.tile_pool(name="persist", bufs=1))

            qkT = [persist.tile([128, S], BF16, tag=f"qkT{i}", name=f"qkT{i}")
                   for i in range(FT)]
            outT = [persist.tile([128, S], BF16, tag=f"outT{h}", name=f"outT{h}")
                    for h in range(QH)]
            vaug = [persist.tile([128, ST * VW], BF16, tag=f"vaug{j}", name=f"vaug{j}")
                    for j in range(KVH)]
            cos_sb = persist.tile([128, S], BF16, tag="cos", name="cos")
            sins_sb = persist.tile([128, S], BF16, tag="sins", name="sins")
            wv_sb = persist.tile([128, KBL * KVH * HD], BF16, tag="wv", name="wv")
            bqk_sb = persist.tile([128, FT], F32, tag="bqk", name="bqk")
            bv_sb = persist.tile([128, KVH * HD], F32, tag="bv", name="bv")
            id_sb = persist.tile([128, 128], BF16, tag="ident", name="ident")

            warm = persist.tile([128, 16], F32, tag="warm", name="warm")
            nc.vector.memset(warm[:], 0.0)
            nc.scalar.activation(warm[:], warm[:],
                                 mybir.ActivationFunctionType.Exp)
            for j in range(KVH):
                nc.vector.memset(vaug[j][:], 1.0)

            # PE HAM warmup: matmuls on a memset tile (no DMA dependency)
            # ride out the ~35us initial weight/activation DMA wait at the
            # cold clock, so the real matmuls start at the full 2.4 GHz
            wmt = persist.tile([128, 128], BF16, tag="wmt", name="wmt")
            nc.vector.memset(wmt[:], 0.25)
            with tc.tile_pool(name="pw", bufs=1, space="PSUM") as pw:
                wps = pw.tile([128, 128], F32, tag="wps", name="wps")
                for _ in range(180):
                    nc.tensor.matmul(wps[:], wmt[:], wmt[:],
                                     start=True, stop=True)

            # ---------------- Phase 1: qkv matmuls + bias + rope ----------------
            # Two f-tile groups: each group's weights load once (no re-reads);
            # k-heads (f=8,9) compute and rope first so attention can begin
            # while the later q-heads still rope.
            def _rope(rp, f):
                rot = rp.tile([128, S], BF16, tag="rot", name="rot")
                nc.sync.dma_start(out=rot[0:64, :], in_=qkT[f][64:128, :])
                nc.sync.dma_start(out=rot[64:128, :], in_=qkT[f][0:64, :])
                t1 = rp.tile([128, S], BF16, tag="t1", name="t1", bufs=1)
                nc.vector.tensor_mul(t1[:], qkT[f][:, :], cos_sb[:, :])
                t2 = rp.tile([128, S], BF16, tag="t2", name="t2", bufs=1)
                nc.vector.tensor_mul(t2[:], rot[:], sins_sb[:, :])
                nc.vector.tensor_add(qkT[f][:, :], t1[:], t2[:])

            rp = ctx.enter_context(tc.tile_pool(name="rope", bufs=2))
            groups = [[8, 9, 0, 1, 2], [3, 4, 5, 6, 7]]
            with tc.tile_pool(name="p1", bufs=2) as p1, \
                 tc.tile_pool(name="ps1", bufs=4, space="PSUM") as ps1:
                for gi, grp in enumerate(groups):
                    # first-needed tiles hit the DMA queues first: w[grp0] and
                    # xt0 split into quarters and interleaved so the first
                    # k-blocks' matmuls can start as early as possible
                    wqs = {}
                    wq_t = p1.tile([128, KBL * 128], BF16, tag="w0",
                                   name="w0", bufs=1)
                    xt0 = p1.tile([128, KBL * SCQ], BF16, tag="xt", name="xt")
                    wq4 = KBL * 128 // 4
                    xq4 = KBL * SCQ // 4
                    for sl in range(4):
                        nc.sync.dma_start(
                            out=wq_t[:, sl * wq4:(sl + 1) * wq4],
                            in_=wqk_e.ap()[grp[0]][:, sl * wq4:(sl + 1) * wq4])
                        nc.sync.dma_start(
                            out=xt0[:, sl * xq4:(sl + 1) * xq4],
                            in_=xt_e.ap()[0][:, sl * xq4:(sl + 1) * xq4])
                    wqs[grp[0]] = wq_t
                    if gi == 1:
                        # needed only from the first rope (early in gi=1)
                        nc.sync.dma_start(out=cos_sb[:], in_=cos_e.ap())
                        nc.sync.dma_start(out=sins_sb[:], in_=sins_e.ap())
                    for i, f in enumerate(grp[1:], start=1):
                        wq_t = p1.tile([128, KBL * 128], BF16, tag=f"w{i}",
                                       name=f"w{i}", bufs=1)
                        nc.sync.dma_start(out=wq_t[:], in_=wqk_e.ap()[f])
                        wqs[f] = wq_t
                        if gi == 0 and i == 1:
                            nc.sync.dma_start(out=bqk_sb[:], in_=bqk_e.ap())
                    if gi == 0:
                        nc.sync.dma_start(out=wv_sb[:], in_=wv_e.ap())
                        nc.sync.dma_start(out=bv_sb[:], in_=bv_e.ap())
                        nc.sync.dma_start(out=id_sb[:], in_=id_e.ap())
                    for c in range(NCQ):
                        if c == 0:
                            xt_t = xt0
                        else:
                            xt_t = p1.tile([128, KBL * SCQ], BF16, tag="xt",
                                           name="xt")
                            nc.sync.dma_start(out=xt_t[:], in_=xt_e.ap()[c])
                        # qk: out[f, s] += wqk[k, f].T @ xT[k, s]
                        for f in grp:
                            psq = ps1.tile([128, SCQ], F32, tag="psq", name="psq")
                            for k in range(KBL):
                                nc.tensor.matmul(
                                    psq[:],
                                    wqs[f][:, k * 128:(k + 1) * 128],
                                    xt_t[:, k * SCQ:(k + 1) * SCQ],
                                    start=(k == 0), stop=(k == KBL - 1))
                            dst = qkT[f][:, c * SCQ:(c + 1) * SCQ]
                            if f < QH:  # fold 1/sqrt(HD) into q
                                nc.vector.tensor_scalar(
                                    dst, psq[:], bqk_sb[:, f:f + 1], SCALE, ADD, MUL)
                            else:
                                nc.vector.tensor_scalar_add(
                                    dst, psq[:], bqk_sb[:, f:f + 1])
                        if gi == 0:
                            # v: out[s, d] += xT[k, s].T @ wv[k, d]
                            for ss in range(SCQ // 128):
                                t_idx = c * (SCQ // 128) + ss
                                psv = ps1.tile([128, KVH * HD], F32, tag="psv",
                                               name="psv")
                                for k in range(KBL):
                                    nc.tensor.matmul(
                                        psv[:],
                                        xt_t[:, k * SCQ + ss * 128:
                                             k * SCQ + ss * 128 + 128],
                                        wv_sb[:, k * KVH * HD:(k + 1) * KVH * HD],
                                        start=(k == 0), stop=(k == KBL - 1))
                                for j in range(KVH):
                                    nc.vector.tensor_add(
                                        vaug[j][:, t_idx * VW: t_idx * VW + HD],
                                        psv[:, j * HD:(j + 1) * HD],
                                        bv_sb[:, j * HD:(j + 1) * HD])
                        # spread group-0's rope across group-1's chunk sweep so
                        # it never head-of-line-blocks the DVE stream
                        if gi == 1 and c < len(groups[0]):
                            _rope(rp, groups[0][c])

            # ---------------- Phase 2: attention ----------------
            # p3 pool opens here so the first c_proj weight tiles can
            # prefetch during attention (kills the phase-2->3 DMA stall)
            p3 = ctx.enter_context(tc.tile_pool(name="p3", bufs=3))

            def _wp_load(ncj):
                wp_t = p3.tile([128, QH * PC], BF16, tag="wp", name=f"wp{ncj}")
                # 4 slices: the kb-chain can start on slice 0
                w4 = QH * PC // 4
                for sl in range(4):
                    nc.sync.dma_start(
                        out=wp_t[:, sl * w4:(sl + 1) * w4],
                        in_=wp_e.ap()[ncj][:, sl * w4:(sl + 1) * w4])
                return wp_t

            with tc.tile_pool(name="p2", bufs=2) as p2, \
                 tc.tile_pool(name="p2n", bufs=6) as p2n, \
                 tc.tile_pool(name="ps_sc", bufs=3, space="PSUM") as ps_sc, \
                 tc.tile_pool(name="ps_av", bufs=2, space="PSUM") as ps_av:
                def _load_mts(qc):
                    mts = {}
                    for k2 in act_pairs(qc):
                        if not pair_needs_mask(qc, k2):
                            continue
                        mt = p2.tile([128, 2 * QC], BF16, tag=f"m{k2}",
                                     name=f"m{k2}", bufs=1)
                        nc.sync.dma_start(out=mt[:], in_=em_e.ap()[qc, k2])
                        mts[k2] = mt
                    return mts

                def _attn_head(qc, h, mts):
                    kv = h // G
                    p_t = p2.tile([128, ST * QC], BF16, tag="p", name="p")
                    for k2 in act_pairs(qc):
                        psc = ps_sc.tile([128, 2 * QC], F32, tag="sc", name="sc")
                        halves = []
                        for half in range(2):
                            kt = 2 * k2 + half
                            jm = jmin(qc, kt)
                            if jm is None:
                                continue
                            off = jm * 128
                            nc.tensor.matmul(
                                psc[:, half * QC + off:(half + 1) * QC],
                                qkT[QH + kv][:, kt * 128:(kt + 1) * 128],
                                qkT[h][:, qc * QC + off:(qc + 1) * QC],
                                start=True, stop=True)
                            halves.append((half, kt, off))
                        pb = p_t[:, k2 * 2 * QC:(k2 + 1) * 2 * QC]
                        if len(halves) == 2 and halves[0][2] == 0 \
                                and halves[1][2] == 0:
                            nc.scalar.activation(pb, psc[:], EXP)
                        else:
                            for half, kt, off in halves:
                                nc.scalar.activation(
                                    pb[:, half * QC + off:(half + 1) * QC],
                                    psc[:, half * QC + off:(half + 1) * QC],
                                    EXP)
                        for half, kt, off in halves:
                            mr = mrange(qc, kt)
                            if mr is None:
                                continue
                            lo = half * QC + mr[0] * 128
                            hi = half * QC + (mr[1] + 1) * 128
                            nc.vector.tensor_mul(
                                pb[:, lo:hi], pb[:, lo:hi], mts[k2][:, lo:hi])
                    for qs in range(NT):
                        kts = [kt for kt in range(ST) if blk(qc, kt, qs)]
                        pav = ps_av.tile([128, VW], F32, tag="av", name="av")
                        for ki, kt in enumerate(kts):
                            nc.tensor.matmul(
                                pav[:],
                                p_t[:, kt * QC + qs * 128: kt * QC + qs * 128 + 128],
                                vaug[kv][:, kt * VW:(kt + 1) * VW],
                                start=(ki == 0), stop=(ki == len(kts) - 1))
                        rc = p2n.tile([128, 1], F32, tag="rc", name="rc")
                        nc.vector.reciprocal(rc[:], pav[:, HD:HD + 1])
                        onrm = p2n.tile([128, 128], BF16, tag="onrm", name="onrm")
                        nc.vector.tensor_scalar_mul(onrm[:], pav[:, 0:HD], rc[:])
                        # XBAR transpose straight into outT: no PE transpose,
                        # no PSUM evacuation copy
                        nc.sync.dma_start_transpose(
                            out=outT[h][:, qc * QC + qs * 128:
                                        qc * QC + qs * 128 + 128],
                            in_=onrm[:])

                # qc=0: interleave group-1's rope between its heads so each
                # head unblocks as soon as its own rope lands
                # descending qc: the densest chunk comes first, so its big
                # heads hide the interleaved one-ahead rope ops
                first = True
                wp_pref = {}
                for qc in range(NQC - 1, -1, -1):
                    mts = _load_mts(qc)
                    if first:
                        # c_proj weight prefetch rides the idle attn DMA window
                        for ncj in range(2):
                            wp_pref[ncj] = _wp_load(ncj)
                    for h in range(QH):
                        if first and h < len(groups[1]):
                            _rope(rp, groups[1][h])
                        _attn_head(qc, h, mts)
                    first = False

            # ---------------- Phase 3: c_proj partial ----------------
            with tc.tile_pool(name="p3o", bufs=6) as p3o, \
                 tc.tile_pool(name="ps3", bufs=6, space="PSUM") as ps3:
                for ncj in range(NPC):
                    wp_t = wp_pref.pop(ncj, None)
                    if wp_t is None:
                        wp_t = _wp_load(ncj)
                    for t in range(ST):
                        pcp = ps3.tile([128, PC], F32, tag="cp", name="cp")
                        for kb in range(QH):
                            nc.tensor.matmul(
                                pcp[:],
                                outT[kb][:, t * 128:(t + 1) * 128],
                                wp_t[:, kb * PC:(kb + 1) * PC],
                                start=(kb == 0), stop=(kb == QH - 1))
                        osb = p3o.tile([128, PC], F32, tag="osb", name="osb")
                        nc.vector.tensor_copy(osb[:], pcp[:])
                        nc.sync.dma_start(
                            out=out_e.ap()[t * 128:(t + 1) * 128,
                                           ncj * PC:(ncj + 1) * PC],
                            in_=osb[:])

    nc.compile()
    return nc


def _prep_core(b, g, hidden_states, attention_mask, em_cache,
               rope_cos, rope_sin, c_attn_w, c_attn_b, c_proj_w, c_proj_b):
    x = hidden_states[b]                                   # [S, H] f32
    xt = x.T.astype(BNP)                                   # [H, S]
    # [NCQ, 128, KBL*SCQ]: xt_t[c, p, k*SCQ+j] = xT[k*128+p, c*SCQ+j]
    xt_t = np.ascontiguousarray(
        xt.reshape(KBL, 128, NCQ, SCQ).transpose(2, 1, 0, 3).reshape(
            NCQ, 128, KBL * SCQ))

    # qk weight columns for this core (f-tiles 0..7 = q heads, 8..9 = k heads)
    cols = []
    for h in range(QH):
        j = 2 * g + h // G
        qi = h % G
        c0 = 768 * j + 128 * qi
        cols.append(np.arange(c0, c0 + 128))
    for lkv in range(KVH):
        j = 2 * g + lkv
        c0 = 768 * j + G * HD
        cols.append(np.arange(c0, c0 + 128))
    cols = np.concatenate(cols)                            # [1280]
    wqk = c_attn_w[:, cols].astype(BNP)                    # [H, 1280]
    # [FT, 128, KBL*128]: wqk_t[f, p, k*128+j] = wqk[k*128+p, f*128+j]
    wqk_t = np.ascontiguousarray(
        wqk.reshape(KBL, 128, FT, 128).transpose(2, 1, 0, 3).reshape(
            FT, 128, KBL * 128))
    bqk = np.ascontiguousarray(
        c_attn_b[cols].astype(np.float32).reshape(FT, 128).T)  # [128, FT]

    vcols = np.concatenate([
        np.arange(768 * (2 * g + lkv) + G * HD + HD,
                  768 * (2 * g + lkv) + G * HD + 2 * HD)
        for lkv in range(KVH)])                            # [256]
    wv = c_attn_w[:, vcols].astype(BNP)                    # [H, 256]
    # [128, KBL*256]: wv_t[p, k*256+j] = wv[k*128+p, j]
    wv_t = np.ascontiguousarray(
        wv.reshape(KBL, 128, KVH * HD).transpose(1, 0, 2).reshape(
            128, KBL * KVH * HD))
    bv = np.ascontiguousarray(np.broadcast_to(
        c_attn_b[vcols].astype(np.float32), (128, KVH * HD)))

    cosT = np.ascontiguousarray(rope_cos.T).astype(BNP)    # [128, S]
    sinT = rope_sin.T.copy()
    sinT[0:64, :] *= -1.0
    sinsT = np.ascontiguousarray(sinT).astype(BNP)

    wp = c_proj_w[1024 * g: 1024 * (g + 1), :].astype(BNP)  # [1024, H]
    # [NPC, 128, QH*PC]: wp_t[n, p, kb*PC+j] = wp[kb*128+p, n*PC+j]
    wp_t = np.ascontiguousarray(
        wp.reshape(QH, 128, NPC, PC).transpose(2, 1, 0, 3).reshape(
            NPC, 128, QH * PC))

    ident = np.eye(128, dtype=BNP)

    return {
        "xt": xt_t, "wqk": wqk_t, "wv": wv_t, "bqk": bqk, "bv": bv,
        "cos": cosT, "sins": sinsT, "emask": em_cache[b], "wp": wp_t,
        "ident": ident,
    }


def _emask(attention_mask, b):
    # exp(maskT) tiled [NQC, ST2, 128, 2*QC]:
    # em[qc, k2, p, t*QC+j] = exp(mask[b,0, qc*QC+j, (2*k2+t)*128+p])
    maskT = attention_mask[b, 0].T                         # [S(ks), S(qs)]
    em = np.exp(maskT, dtype=np.float32)
    em_t = np.ascontiguousarray(
        em.reshape(ST2, 2, 128, NQC, QC).transpose(3, 0, 2, 1, 4).reshape(
            NQC, ST2, 128, 2 * QC)).astype(BNP)
    return em_t


def _classify(em_cache):
    # per-block class over ALL batches (one SPMD graph serves every core):
    # 0 = exp(mask) all-zero in every batch, 1 = all-one in every batch,
    # 2 = anything else
    one = np.float32(1.0)
    pat = []
    for qc in range(NQC):
        row = []
        for kt in range(ST):
            k2, half = divmod(kt, 2)
            blocks = []
            for j in range(NT):
                cls = None
                for b in range(B):
                    t = em_cache[b][qc, k2][:, half * QC + j * 128:
                                            half * QC + (j + 1) * 128]
                    if not np.any(t):
                        c = 0
                    elif np.all(t == one):
                        c = 1
                    else:
                        c = 2
                    cls = c if cls is None else (cls if cls == c else 2)
                blocks.append(cls)
            row.append(tuple(blocks))
        pat.append(tuple(row))
    return tuple(pat)


def _rope_np(x, cos, sin):
    h = HD // 2
    x1, x2 = x[..., :h], x[..., h:]
    rot = np.concatenate([-x2, x1], axis=-1)
    return x * cos[None, None] + rot * sin[None, None]


def _kernel_numpy(hidden_states, attention_mask, rope_cos, rope_sin,
                  c_attn_w, c_attn_b, c_proj_w, c_proj_b):
    """Exact reference math in numpy: stability fallback for score regimes
    outside fp32-exp range (never triggers on sanely-scaled inputs)."""
    qkv = hidden_states @ c_attn_w + c_attn_b
    qkv = qkv.reshape(B, S, NKV, G * HD + 2 * HD)
    q = qkv[..., :G * HD].reshape(B, S, NH, HD).transpose(0, 2, 1, 3)
    k = qkv[..., G * HD:G * HD + HD].transpose(0, 2, 1, 3)
    v = qkv[..., G * HD + HD:].transpose(0, 2, 1, 3)
    q = _rope_np(q, rope_cos, rope_sin)
    k = _rope_np(k, rope_cos, rope_sin)
    k = np.repeat(k, G, axis=1)
    v = np.repeat(v, G, axis=1)
    out = np.empty((B, NH, S, HD), np.float32)
    for b in range(B):
        for h in range(NH):
            s_ = (q[b, h] @ k[b, h].T) * SCALE + attention_mask[b, 0]
            s_ = s_ - s_.max(axis=-1, keepdims=True)
            p = np.exp(s_, dtype=np.float32)
            out[b, h] = (p / p.sum(axis=-1, keepdims=True)) @ v[b, h]
    out = out.transpose(0, 2, 1, 3).reshape(B, S, H)
    return out @ c_proj_w + c_proj_b


def _score_scale_probe(hidden_states, attention_mask, rope_cos, rope_sin,
                       c_attn_w, c_attn_b):
    """Upper estimate of max |score + mask| via a small exact sample."""
    x = hidden_states[0, :256]                      # [256, H]
    j = 0
    qc = c_attn_w[:, 768 * j:768 * j + 128]
    kc = c_attn_w[:, 768 * j + 512:768 * j + 640]
    q = (x[:32] @ qc + c_attn_b[768 * j:768 * j + 128])[None, None]
    k = (x @ kc + c_attn_b[768 * j + 512:768 * j + 640])[None, None]
    q = _rope_np(q, rope_cos[:32], rope_sin[:32])[0, 0]
    k = _rope_np(k, rope_cos[:256], rope_sin[:256])[0, 0]
    s_ = (q @ k.T) * SCALE
    m = attention_mask[0, 0, :32, :256]
    pos = np.abs(s_).std() * 8.0 + max(0.0, float(m.max()))
    return pos


def kernel(hidden_states, attention_mask, rope_cos, rope_sin,
           c_attn_w, c_attn_b, c_proj_w, c_proj_b):
    global LAST_EXEC_NS, LAST_RESULTS
    hidden_states = np.asarray(hidden_states, dtype=np.float32)
    attention_mask = np.asarray(attention_mask, dtype=np.float32)
    rope_cos = np.asarray(rope_cos, dtype=np.float32)
    rope_sin = np.asarray(rope_sin, dtype=np.float32)
    c_attn_w = np.asarray(c_attn_w, dtype=np.float32)
    c_attn_b = np.asarray(c_attn_b, dtype=np.float32)
    c_proj_w = np.asarray(c_proj_w, dtype=np.float32)
    c_proj_b = np.asarray(c_proj_b, dtype=np.float32)

    if _score_scale_probe(hidden_states, attention_mask, rope_cos,
                          rope_sin, c_attn_w, c_attn_b) > 75.0:
        # scores would overflow fp32 exp without per-row max subtraction;
        # use the exact (slow) host path rather than returning garbage
        LAST_EXEC_NS = None
        return _kernel_numpy(hidden_states, attention_mask, rope_cos,
                             rope_sin, c_attn_w, c_attn_b, c_proj_w,
                             c_proj_b)

    em_cache = [_emask(attention_mask, b) for b in range(B)]
    pattern = _classify(em_cache)
    # safety: every (qc, qs-subtile) needs at least one contributing ks
    # block, else softmax Z would be empty -> fall back to dense pattern
    degenerate = any(
        not any(pattern[qc][kt][j] for kt in range(ST))
        for qc in range(NQC) for j in range(NT))
    if degenerate:
        pattern = tuple(
            tuple(tuple(2 for _ in range(NT)) for _ in range(ST))
            for _ in range(NQC))

    if pattern not in _CACHE:
        _CACHE[pattern] = _build_nc(pattern)
    nc = _CACHE[pattern]
    in_maps = []
    for core in range(8):
        b, g = divmod(core, 4)
        in_maps.append(_prep_core(b, g, hidden_states, attention_mask, em_cache,
                                  rope_cos, rope_sin, c_attn_w, c_attn_b,
                                  c_proj_w, c_proj_b))

    trace = bool(int(os.environ.get("BASS_KERNEL_TRACE", "0")))
    res = run_bass_kernel_spmd(nc, in_maps, list(range(8)), trace=trace)
    LAST_EXEC_NS = res.exec_time_ns
    LAST_RESULTS = res

    out = np.zeros((B, S, H), dtype=np.float32)
    for core in range(8):
        b = core // 4
        out[b] += res.results[core]["out"]
    out += c_proj_b[None, None, :]
    return out


# revision 9
# speedup vs baseline: 1.0507x; 1.0507x over previous
"""Distributed GQA attention block for TRN2 (8 NeuronCores).

Sharding: core = b*4 + g  (b = batch 0..1, g = kv-head-pair 0..3).
Each core computes qkv for its 8 q-heads / 2 kv-heads, full attention for
those heads, and a partial c_proj ([2048,4096]); host sums the 4 partials
per batch and adds c_proj bias.

All PE-facing tensors are bf16 (fp32 PSUM accumulation). Softmax runs
without max-subtraction (scores are O(30), safe in fp32 exp), and the
additive mask is applied as exp(s+m) = exp(s)*exp(m) with exp(m)
precomputed on host, so ScalarE exps raw PSUM scores directly.

The mask is classified per 128x128 block (skip / identity / general), so
for a causal mask the score+exp+AV work shrinks to the lower-triangular
blocks and the exp(m) multiply runs only on diagonal-crossing blocks.
"""
import sys, os, types

sys.path.insert(0, '/opt/trn_rl_repo')

# Inject the NTFF profile hook module that this image's antenv lacks
# (needed only when tracing; harmless otherwise).
try:
    import antenv
    if "antenv.axon_hooks" not in sys.modules:
        _m = types.ModuleType("antenv.axon_hooks")
        _m._hook = None
        def _set(h, _m=_m): _m._hook = h
        def _get(_m=_m): return _m._hook
        _m.set_axon_ntff_profile_hook = _set
        _m.get_axon_ntff_profile_hook = _get
        sys.modules["antenv.axon_hooks"] = _m
        antenv.axon_hooks = _m
        try:
            from trn_agent_boot.trn_boot import _ntff_profile_via_ctypes
            _set(_ntff_profile_via_ctypes('/opt/axon/libaxon_pjrt.so'))
        except Exception:
            pass
except Exception:
    pass

import numpy as np
import ml_dtypes

import concourse.bass as bass
import concourse.tile as tile
from concourse import bacc, mybir
from concourse.bass_utils import run_bass_kernel_spmd

BF16 = mybir.dt.bfloat16
F32 = mybir.dt.float32
BNP = ml_dtypes.bfloat16

B, S, H = 2, 2048, 4096
NH, NKV, HD = 32, 8, 128
G = NH // NKV                  # 4 q heads per kv head
QH = 8                         # q heads per core
KVH = 2                        # kv heads per core
FT = QH + KVH                  # 10 qk feature tiles per core
ST = S // 128                  # 16 s tiles
KBL = H // 128                 # 32 contraction blocks
SCQ = 256                      # qkv-phase seq chunk
NCQ = S // SCQ                 # 8
QC = 512                       # attention qs chunk
NQC = S // QC                  # 4
NT = QC // 128                 # 4 qs subtiles per chunk
ST2 = ST // 2                  # paired score-tile groups (1024-wide psum)
PC = 512                       # c_proj n chunk
NPC = H // PC                  # 8
SCALE = 1.0 / float(np.sqrt(HD))
VW = HD + 1                    # v-aug row width (ones column for softmax Z)

_CACHE = {}
LAST_EXEC_NS = None
LAST_RESULTS = None


def _build_nc(pattern):
    # pattern[qc][kt][j] classifies the [128 ks x 128 qs] block
    # (ks tile kt, qs subtile j of chunk qc) of exp(mask):
    #   0 => identically zero: block skipped entirely (exact).
    #   1 => identically one: computed, no mask multiply.
    #   2 => general: computed, multiplied by exp(mask).
    nc = bacc.Bacc("TRN2", target_bir_lowering=False, debug=False, num_devices=8)

    xt_e = nc.declare_dram_parameter("xt", [NCQ, 128, KBL * SCQ], BF16, isOutput=False)
    wqk_e = nc.declare_dram_parameter("wqk", [FT, 128, KBL * 128], BF16, isOutput=False)
    wv_e = nc.declare_dram_parameter("wv", [128, KBL * KVH * HD], BF16, isOutput=False)
    bqk_e = nc.declare_dram_parameter("bqk", [128, FT], F32, isOutput=False)
    bv_e = nc.declare_dram_parameter("bv", [128, KVH * HD], F32, isOutput=False)
    cos_e = nc.declare_dram_parameter("cos", [128, S], BF16, isOutput=False)
    sins_e = nc.declare_dram_parameter("sins", [128, S], BF16, isOutput=False)
    em_e = nc.declare_dram_parameter("emask", [NQC, ST2, 128, 2 * QC], BF16,
                                     isOutput=False)
    wp_e = nc.declare_dram_parameter("wp", [NPC, 128, QH * PC], BF16, isOutput=False)
    id_e = nc.declare_dram_parameter("ident", [128, 128], BF16, isOutput=False)
    out_e = nc.declare_dram_parameter("out", [S, H], F32, isOutput=True)

    ADD = mybir.AluOpType.add
    MUL = mybir.AluOpType.mult
    EXP = mybir.ActivationFunctionType.Exp

    # ---- pattern-derived helpers (all build-time constants) ----
    def blk(qc, kt, j):
        return pattern[qc][kt][j]

    def jmin(qc, kt):
        act = [j for j in range(NT) if blk(qc, kt, j)]
        return act[0] if act else None

    def mrange(qc, kt):
        # qs-subtile range needing the exp(mask) multiply: class-2 blocks,
        # plus class-0 blocks sitting inside the computed range (their
        # exp(mask)=0 zeroes the computed scores).
        jm = jmin(qc, kt)
        if jm is None:
            return None
        need = [j for j in range(jm, NT) if blk(qc, kt, j) != 1]
        if not need:
            return None
        return (need[0], need[-1])

    def act_pairs(qc):
        return [k2 for k2 in range(ST2)
                if jmin(qc, 2 * k2) is not None or jmin(qc, 2 * k2 + 1) is not None]

    def pair_needs_mask(qc, k2):
        return (mrange(qc, 2 * k2) is not None or
                mrange(qc, 2 * k2 + 1) is not None)

    with tile.TileContext(nc) as tc:
        from contextlib import ExitStack
        with ExitStack() as ctx:
            persist = ctx.enter_context(tc.tile_pool(name="persist", bufs=1))

            qkT = [persist.tile([128, S], BF16, tag=f"qkT{i}", name=f"qkT{i}")
                   for i in range(FT)]
            outT = [persist.tile([128, S], BF16, tag=f"outT{h}", name=f"outT{h}")
                    for h in range(QH)]
            vaug = [persist.tile([128, ST * VW], BF16, tag=f"vaug{j}", name=f"vaug{j}")
                    for j in range(KVH)]
            cos_sb = persist.tile([128, S], BF16, tag="cos", name="cos")
            sins_sb = persist.tile([128, S], BF16, tag="sins", name="sins")
            wv_sb = persist.tile([128, KBL * KVH * HD], BF16, tag="wv", name="wv")
            bqk_sb = persist.tile([128, FT], F32, tag="bqk", name="bqk")
            bv_sb = persist.tile([128, KVH * HD], F32, tag="bv", name="bv")
            id_sb = persist.tile([128, 128], BF16, tag="ident", name="ident")

            warm = persist.tile([128, 16], F32, tag="warm", name="warm")
            nc.vector.memset(warm[:], 0.0)
            nc.scalar.activation(warm[:], warm[:],
                                 mybir.ActivationFunctionType.Exp)
            for j in range(KVH):
                nc.vector.memset(vaug[j][:], 1.0)

            # PE HAM warmup: matmuls on a memset tile (no DMA dependency)
            # ride out the ~35us initial weight/activation DMA wait at the
            # cold clock, so the real matmuls start at the full 2.4 GHz
            wmt = persist.tile([128, 128], BF16, tag="wmt", name="wmt")
            nc.vector.memset(wmt[:], 0.25)
            with tc.tile_pool(name="pw", bufs=1, space="PSUM") as pw:
                wps = pw.tile([128, 128], F32, tag="wps", name="wps")
                for _ in range(180):
                    nc.tensor.matmul(wps[:], wmt[:], wmt[:],
                                     start=True, stop=True)

            # ---------------- Phase 1: qkv matmuls + bias + rope ----------------
            # Two f-tile groups: each group's weights load once (no re-reads);
            # k-heads (f=8,9) compute and rope first so attention can begin
            # while the later q-heads still rope.
            def _rope(rp, f):
                rot = rp.tile([128, S], BF16, tag="rot", name="rot")
                nc.sync.dma_start(out=rot[0:64, :], in_=qkT[f][64:128, :])
                nc.sync.dma_start(out=rot[64:128, :], in_=qkT[f][0:64, :])
                t1 = rp.tile([128, S], BF16, tag="t1", name="t1", bufs=1)
                nc.vector.tensor_mul(t1[:], qkT[f][:, :], cos_sb[:, :])
                t2 = rp.tile([128, S], BF16, tag="t2", name="t2", bufs=1)
                nc.vector.tensor_mul(t2[:], rot[:], sins_sb[:, :])
                nc.vector.tensor_add(qkT[f][:, :], t1[:], t2[:])

            rp = ctx.enter_context(tc.tile_pool(name="rope", bufs=2))
            groups = [[8, 9, 0, 1, 2], [3, 4, 5, 6, 7]]
            with tc.tile_pool(name="p1", bufs=2) as p1, \
                 tc.tile_pool(name="ps1", bufs=4, space="PSUM") as ps1:
                for gi, grp in enumerate(groups):
                    # first-needed tiles hit the DMA queues first: w[grp0] and
                    # xt0 split into quarters and interleaved so the first
                    # k-blocks' matmuls can start as early as possible
                    wqs = {}
                    wq_t = p1.tile([128, KBL * 128], BF16, tag="w0",
                                   name="w0", bufs=1)
                    xt0 = p1.tile([128, KBL * SCQ], BF16, tag="xt", name="xt")
                    wq4 = KBL * 128 // 4
                    xq4 = KBL * SCQ // 4
                    for sl in range(4):
                        nc.sync.dma_start(
                            out=wq_t[:, sl * wq4:(sl + 1) * wq4],
                            in_=wqk_e.ap()[grp[0]][:, sl * wq4:(sl + 1) * wq4])
                        nc.sync.dma_start(
                            out=xt0[:, sl * xq4:(sl + 1) * xq4],
                            in_=xt_e.ap()[0][:, sl * xq4:(sl + 1) * xq4])
                    wqs[grp[0]] = wq_t
                    if gi == 1:
                        # needed only from the first rope (early in gi=1)
                        nc.sync.dma_start(out=cos_sb[:], in_=cos_e.ap())
                        nc.sync.dma_start(out=sins_sb[:], in_=sins_e.ap())
                    for i, f in enumerate(grp[1:], start=1):
                        wq_t = p1.tile([128, KBL * 128], BF16, tag=f"w{i}",
                                       name=f"w{i}", bufs=1)
                        nc.sync.dma_start(out=wq_t[:], in_=wqk_e.ap()[f])
                        wqs[f] = wq_t
                        if gi == 0 and i == 1:
                            nc.sync.dma_start(out=bqk_sb[:], in_=bqk_e.ap())
                    if gi == 0:
                        nc.sync.dma_start(out=wv_sb[:], in_=wv_e.ap())
                        nc.sync.dma_start(out=bv_sb[:], in_=bv_e.ap())
                        nc.sync.dma_start(out=id_sb[:], in_=id_e.ap())
                    for c in range(NCQ):
                        if c == 0:
                            xt_t = xt0
                        else:
                            xt_t = p1.tile([128, KBL * SCQ], BF16, tag="xt",
                                           name="xt")
                            nc.sync.dma_start(out=xt_t[:], in_=xt_e.ap()[c])
                        # qk: out[f, s] += wqk[k, f].T @ xT[k, s]
                        for f in grp:
                            psq = ps1.tile([128, SCQ], F32, tag="psq", name="psq")
                            for k in range(KBL):
                                nc.tensor.matmul(
                                    psq[:],
                                    wqs[f][:, k * 128:(k + 1) * 128],
                                    xt_t[:, k * SCQ:(k + 1) * SCQ],
                                    start=(k == 0), stop=(k == KBL - 1))
                            dst = qkT[f][:, c * SCQ:(c + 1) * SCQ]
                            if f < QH:  # fold 1/sqrt(HD) into q
                                nc.vector.tensor_scalar(
                                    dst, psq[:], bqk_sb[:, f:f + 1], SCALE, ADD, MUL)
                            else:
                                nc.vector.tensor_scalar_add(
                                    dst, psq[:], bqk_sb[:, f:f + 1])
                        if gi == 0:
                            # v: out[s, d] += xT[k, s].T @ wv[k, d]
                            for ss in range(SCQ // 128):
                                t_idx = c * (SCQ // 128) + ss
                                psv = ps1.tile([128, KVH * HD], F32, tag="psv",
                                               name="psv")
                                for k in range(KBL):
                                    nc.tensor.matmul(
                                        psv[:],
                                        xt_t[:, k * SCQ + ss * 128:
                                             k * SCQ + ss * 128 + 128],
                                        wv_sb[:, k * KVH * HD:(k + 1) * KVH * HD],
                                        start=(k == 0), stop=(k == KBL - 1))
                                for j in range(KVH):
                                    nc.vector.tensor_add(
                                        vaug[j][:, t_idx * VW: t_idx * VW + HD],
                                        psv[:, j * HD:(j + 1) * HD],
                                        bv_sb[:, j * HD:(j + 1) * HD])
                        # spread group-0's rope across group-1's chunk sweep so
                        # it never head-of-line-blocks the DVE stream
                        if gi == 1 and c < len(groups[0]):
                            _rope(rp, groups[0][c])

            # ---------------- Phase 2+3: attention with c_proj interleave ----
            # c_proj pools open alongside attention: wp weights prefetch on
            # the idle attn DMA window and proj matmul chains are emitted at
            # qc boundaries to fill PE slack while ScalarE exps (the attn
            # bottleneck) keep streaming.
            p3 = ctx.enter_context(tc.tile_pool(name="p3", bufs=3))
            p3o = ctx.enter_context(tc.tile_pool(name="p3o", bufs=3))
            ps3 = ctx.enter_context(tc.tile_pool(name="ps3", bufs=2,
                                                 space="PSUM"))

            def _wp_load(ncj):
                wp_t = p3.tile([128, QH * PC], BF16, tag="wp", name=f"wp{ncj}")
                # 4 slices: the kb-chain can start on slice 0
                w4 = QH * PC // 4
                for sl in range(4):
                    nc.sync.dma_start(
                        out=wp_t[:, sl * w4:(sl + 1) * w4],
                        in_=wp_e.ap()[ncj][:, sl * w4:(sl + 1) * w4])
                return wp_t

            def _proj(ncj, ts, wp_t):
                for t in ts:
                    pcp = ps3.tile([128, PC], F32, tag="cp", name="cp")
                    for kb in range(QH):
                        nc.tensor.matmul(
                            pcp[:],
                            outT[kb][:, t * 128:(t + 1) * 128],
                            wp_t[:, kb * PC:(kb + 1) * PC],
                            start=(kb == 0), stop=(kb == QH - 1))
                    osb = p3o.tile([128, PC], F32, tag="osb", name="osb")
                    nc.vector.tensor_copy(osb[:], pcp[:])
                    nc.sync.dma_start(
                        out=out_e.ap()[t * 128:(t + 1) * 128,
                                       ncj * PC:(ncj + 1) * PC],
                        in_=osb[:])

            with tc.tile_pool(name="p2", bufs=2) as p2, \
                 tc.tile_pool(name="p2n", bufs=4) as p2n, \
                 tc.tile_pool(name="ps_sc", bufs=2, space="PSUM") as ps_sc, \
                 tc.tile_pool(name="ps_av", bufs=2, space="PSUM") as ps_av:
                def _load_mts(qc):
                    mts = {}
                    for k2 in act_pairs(qc):
                        if not pair_needs_mask(qc, k2):
                            continue
                        mt = p2.tile([128, 2 * QC], BF16, tag=f"m{k2}",
                                     name=f"m{k2}", bufs=1)
                        nc.sync.dma_start(out=mt[:], in_=em_e.ap()[qc, k2])
                        mts[k2] = mt
                    return mts

                def _emit_scores(qc, h, mts):
                    kv = h // G
                    p_t = p2.tile([128, ST * QC], BF16, tag="p", name="p")
                    for k2 in act_pairs(qc):
                        psc = ps_sc.tile([128, 2 * QC], F32, tag="sc", name="sc")
                        halves = []
                        for half in range(2):
                            kt = 2 * k2 + half
                            jm = jmin(qc, kt)
                            if jm is None:
                                continue
                            off = jm * 128
                            nc.tensor.matmul(
                                psc[:, half * QC + off:(half + 1) * QC],
                                qkT[QH + kv][:, kt * 128:(kt + 1) * 128],
                                qkT[h][:, qc * QC + off:(qc + 1) * QC],
                                start=True, stop=True)
                            halves.append((half, kt, off))
                        pb = p_t[:, k2 * 2 * QC:(k2 + 1) * 2 * QC]
                        spans = [(half * QC + off, (half + 1) * QC)
                                 for half, kt, off in halves]
                        lo = min(s for s, e in spans)
                        hi = max(e for s, e in spans)
                        useful = sum(e - s for s, e in spans)
                        if (hi - lo) - useful <= 256:
                            # one ACTIVATE (352-cycle fixed cost dominates):
                            # any gap is exp'd garbage that nothing reads
                            nc.scalar.activation(pb[:, lo:hi], psc[:, lo:hi],
                                                 EXP)
                        else:
                            for s, e in spans:
                                nc.scalar.activation(pb[:, s:e], psc[:, s:e],
                                                     EXP)
                        for half, kt, off in halves:
                            mr = mrange(qc, kt)
                            if mr is None:
                                continue
                            ml = half * QC + mr[0] * 128
                            mh = half * QC + (mr[1] + 1) * 128
                            nc.vector.tensor_mul(
                                pb[:, ml:mh], pb[:, ml:mh], mts[k2][:, ml:mh])
                    return p_t

                def _emit_av(qc, h, p_t):
                    kv = h // G
                    # staging holds the 4 normalized [qs,d] subtiles side by
                    # side; one XBAR transpose (out[d, j, q] = stg[q, j*128+d])
                    # moves the whole [QC, HD] block into outT
                    stg = p2n.tile([128, QC], BF16, tag="stg", name="stg")
                    for qs in range(NT):
                        kts = [kt for kt in range(ST) if blk(qc, kt, qs)]
                        pav = ps_av.tile([128, VW], F32, tag="av", name="av")
                        for ki, kt in enumerate(kts):
                            nc.tensor.matmul(
                                pav[:],
                                p_t[:, kt * QC + qs * 128:
                                    kt * QC + qs * 128 + 128],
                                vaug[kv][:, kt * VW:(kt + 1) * VW],
                                start=(ki == 0), stop=(ki == len(kts) - 1))
                        rc = p2n.tile([128, 1], F32, tag="rc", name="rc")
                        nc.vector.reciprocal(rc[:], pav[:, HD:HD + 1])
                        nc.vector.tensor_scalar_mul(
                            stg[:, qs * 128:(qs + 1) * 128],
                            pav[:, 0:HD], rc[:])
                    nc.sync.dma_start_transpose(
                        out=outT[h][:, qc * QC:(qc + 1) * QC].rearrange(
                            "p (j q) -> p j q", j=NT),
                        in_=stg[:])

                # software pipeline: scores(h+1) is emitted before AV(h) so
                # the PE never head-of-line blocks on ScalarE's exp of head h.
                # descending qc: the densest chunk comes first; proj chains
                # for finished qc's t-tiles fill PE slack from then on.
                first = True
                wp_tiles = {}
                pend = [None]

                def _flush():
                    if pend[0] is not None:
                        _emit_av(*pend[0])
                        pend[0] = None

                for qc in range(NQC - 1, -1, -1):
                    mts = _load_mts(qc)
                    if first:
                        # c_proj weight prefetch rides the idle attn DMA window
                        wp_tiles[0] = _wp_load(0)
                        wp_tiles[1] = _wp_load(1)
                    for h in range(QH):
                        if first and h < len(groups[1]):
                            _rope(rp, groups[1][h])
                        p_t = _emit_scores(qc, h, mts)
                        _flush()
                        pend[0] = (qc, h, p_t)
                    first = False
                    _flush()
                    # proj chains over completed qc's t-tiles
                    if qc == 3:
                        _proj(0, range(12, 16), wp_tiles[0])
                    elif qc == 2:
                        _proj(0, range(8, 12), wp_tiles[0])
                        _proj(1, range(8, 16), wp_tiles[1])
                    elif qc == 1:
                        _proj(0, range(4, 8), wp_tiles[0])
                        _proj(1, range(4, 8), wp_tiles[1])
                        wp_tiles[2] = _wp_load(2)
                        _proj(2, range(4, 16), wp_tiles[2])
                    else:
                        _proj(0, range(0, 4), wp_tiles[0])
                        _proj(1, range(0, 4), wp_tiles[1])
                        _proj(2, range(0, 4), wp_tiles[2])
                        for ncj in range(3, NPC):
                            wp_t = _wp_load(ncj)
                            _proj(ncj, range(ST), wp_t)

    nc.compile()
    return nc


def _prep_core(b, g, hidden_states, attention_mask, em_cache,
               rope_cos, rope_sin, c_attn_w, c_attn_b, c_proj_w, c_proj_b):
    x = hidden_states[b]                                   # [S, H] f32
    xt = x.T.astype(BNP)                                   # [H, S]
    # [NCQ, 128, KBL*SCQ]: xt_t[c, p, k*SCQ+j] = xT[k*128+p, c*SCQ+j]
    xt_t = np.ascontiguousarray(
        xt.reshape(KBL, 128, NCQ, SCQ).transpose(2, 1, 0, 3).reshape(
            NCQ, 128, KBL * SCQ))

    # qk weight columns for this core (f-tiles 0..7 = q heads, 8..9 = k heads)
    cols = []
    for h in range(QH):
        j = 2 * g + h // G
        qi = h % G
        c0 = 768 * j + 128 * qi
        cols.append(np.arange(c0, c0 + 128))
    for lkv in range(KVH):
        j = 2 * g + lkv
        c0 = 768 * j + G * HD
        cols.append(np.arange(c0, c0 + 128))
    cols = np.concatenate(cols)                            # [1280]
    wqk = c_attn_w[:, cols].astype(BNP)                    # [H, 1280]
    # [FT, 128, KBL*128]: wqk_t[f, p, k*128+j] = wqk[k*128+p, f*128+j]
    wqk_t = np.ascontiguousarray(
        wqk.reshape(KBL, 128, FT, 128).transpose(2, 1, 0, 3).reshape(
            FT, 128, KBL * 128))
    bqk = np.ascontiguousarray(
        c_attn_b[cols].astype(np.float32).reshape(FT, 128).T)  # [128, FT]

    vcols = np.concatenate([
        np.arange(768 * (2 * g + lkv) + G * HD + HD,
                  768 * (2 * g + lkv) + G * HD + 2 * HD)
        for lkv in range(KVH)])                            # [256]
    wv = c_attn_w[:, vcols].astype(BNP)                    # [H, 256]
    # [128, KBL*256]: wv_t[p, k*256+j] = wv[k*128+p, j]
    wv_t = np.ascontiguousarray(
        wv.reshape(KBL, 128, KVH * HD).transpose(1, 0, 2).reshape(
            128, KBL * KVH * HD))
    bv = np.ascontiguousarray(np.broadcast_to(
        c_attn_b[vcols].astype(np.float32), (128, KVH * HD)))

    cosT = np.ascontiguousarray(rope_cos.T).astype(BNP)    # [128, S]
    sinT = rope_sin.T.copy()
    sinT[0:64, :] *= -1.0
    sinsT = np.ascontiguousarray(sinT).astype(BNP)

    wp = c_proj_w[1024 * g: 1024 * (g + 1), :].astype(BNP)  # [1024, H]
    # [NPC, 128, QH*PC]: wp_t[n, p, kb*PC+j] = wp[kb*128+p, n*PC+j]
    wp_t = np.ascontiguousarray(
        wp.reshape(QH, 128, NPC, PC).transpose(2, 1, 0, 3).reshape(
            NPC, 128, QH * PC))

    ident = np.eye(128, dtype=BNP)

    return {
        "xt": xt_t, "wqk": wqk_t, "wv": wv_t, "bqk": bqk, "bv": bv,
        "cos": cosT, "sins": sinsT, "emask": em_cache[b], "wp": wp_t,
        "ident": ident,
    }


def _emask(attention_mask, b):
    # exp(maskT) tiled [NQC, ST2, 128, 2*QC]:
    # em[qc, k2, p, t*QC+j] = exp(mask[b,0, qc*QC+j, (2*k2+t)*128+p])
    maskT = attention_mask[b, 0].T                         # [S(ks), S(qs)]
    em = np.exp(maskT, dtype=np.float32)
    em_t = np.ascontiguousarray(
        em.reshape(ST2, 2, 128, NQC, QC).transpose(3, 0, 2, 1, 4).reshape(
            NQC, ST2, 128, 2 * QC)).astype(BNP)
    return em_t


def _classify(em_cache):
    # per-block class over ALL batches (one SPMD graph serves every core):
    # 0 = exp(mask) all-zero in every batch, 1 = all-one in every batch,
    # 2 = anything else
    one = np.float32(1.0)
    pat = []
    for qc in range(NQC):
        row = []
        for kt in range(ST):
            k2, half = divmod(kt, 2)
            blocks = []
            for j in range(NT):
                cls = None
                for b in range(B):
                    t = em_cache[b][qc, k2][:, half * QC + j * 128:
                                            half * QC + (j + 1) * 128]
                    if not np.any(t):
                        c = 0
                    elif np.all(t == one):
                        c = 1
                    else:
                        c = 2
                    cls = c if cls is None else (cls if cls == c else 2)
                blocks.append(cls)
            row.append(tuple(blocks))
        pat.append(tuple(row))
    return tuple(pat)


def _rope_np(x, cos, sin):
    h = HD // 2
    x1, x2 = x[..., :h], x[..., h:]
    rot = np.concatenate([-x2, x1], axis=-1)
    return x * cos[None, None] + rot * sin[None, None]


def _kernel_numpy(hidden_states, attention_mask, rope_cos, rope_sin,
                  c_attn_w, c_attn_b, c_proj_w, c_proj_b):
    """Exact reference math in numpy: stability fallback for score regimes
    outside fp32-exp range (never triggers on sanely-scaled inputs)."""
    qkv = hidden_states @ c_attn_w + c_attn_b
    qkv = qkv.reshape(B, S, NKV, G * HD + 2 * HD)
    q = qkv[..., :G * HD].reshape(B, S, NH, HD).transpose(0, 2, 1, 3)
    k = qkv[..., G * HD:G * HD + HD].transpose(0, 2, 1, 3)
    v = qkv[..., G * HD + HD:].transpose(0, 2, 1, 3)
    q = _rope_np(q, rope_cos, rope_sin)
    k = _rope_np(k, rope_cos, rope_sin)
    k = np.repeat(k, G, axis=1)
    v = np.repeat(v, G, axis=1)
    out = np.empty((B, NH, S, HD), np.float32)
    for b in range(B):
        for h in range(NH):
            s_ = (q[b, h] @ k[b, h].T) * SCALE + attention_mask[b, 0]
            s_ = s_ - s_.max(axis=-1, keepdims=True)
            p = np.exp(s_, dtype=np.float32)
            out[b, h] = (p / p.sum(axis=-1, keepdims=True)) @ v[b, h]
    out = out.transpose(0, 2, 1, 3).reshape(B, S, H)
    return out @ c_proj_w + c_proj_b


def _score_scale_probe(hidden_states, attention_mask, rope_cos, rope_sin,
                       c_attn_w, c_attn_b):
    """Upper estimate of max |score + mask| via a small exact sample."""
    x = hidden_states[0, :256]                      # [256, H]
    j = 0
    qc = c_attn_w[:, 768 * j:768 * j + 128]
    kc = c_attn_w[:, 768 * j + 512:768 * j + 640]
    q = (x[:32] @ qc + c_attn_b[768 * j:768 * j + 128])[None, None]
    k = (x @ kc + c_attn_b[768 * j + 512:768 * j + 640])[None, None]
    q = _rope_np(q, rope_cos[:32], rope_sin[:32])[0, 0]
    k = _rope_np(k, rope_cos[:256], rope_sin[:256])[0, 0]
    s_ = (q @ k.T) * SCALE
    m = attention_mask[0, 0, :32, :256]
    pos = np.abs(s_).std() * 8.0 + max(0.0, float(m.max()))
    return pos


def kernel(hidden_states, attention_mask, rope_cos, rope_sin,
           c_attn_w, c_attn_b, c_proj_w, c_proj_b):
    global LAST_EXEC_NS, LAST_RESULTS
    hidden_states = np.asarray(hidden_states, dtype=np.float32)
    attention_mask = np.asarray(attention_mask, dtype=np.float32)
    rope_cos = np.asarray(rope_cos, dtype=np.float32)
    rope_sin = np.asarray(rope_sin, dtype=np.float32)
    c_attn_w = np.asarray(c_attn_w, dtype=np.float32)
    c_attn_b = np.asarray(c_attn_b, dtype=np.float32)
    c_proj_w = np.asarray(c_proj_w, dtype=np.float32)
    c_proj_b = np.asarray(c_proj_b, dtype=np.float32)

    if _score_scale_probe(hidden_states, attention_mask, rope_cos,
                          rope_sin, c_attn_w, c_attn_b) > 75.0:
        # scores would overflow fp32 exp without per-row max subtraction;
        # use the exact (slow) host path rather than returning garbage
        LAST_EXEC_NS = None
        return _kernel_numpy(hidden_states, attention_mask, rope_cos,
                             rope_sin, c_attn_w, c_attn_b, c_proj_w,
                             c_proj_b)

    em_cache = [_emask(attention_mask, b) for b in range(B)]
    pattern = _classify(em_cache)
    # safety: every (qc, qs-subtile) needs at least one contributing ks
    # block, else softmax Z would be empty -> fall back to dense pattern
    degenerate = any(
        not any(pattern[qc][kt][j] for kt in range(ST))
        for qc in range(NQC) for j in range(NT))
    if degenerate:
        pattern = tuple(
            tuple(tuple(2 for _ in range(NT)) for _ in range(ST))
            for _ in range(NQC))

    if pattern not in _CACHE:
        _CACHE[pattern] = _build_nc(pattern)
    nc = _CACHE[pattern]
    in_maps = []
    for core in range(8):
        b, g = divmod(core, 4)
        in_maps.append(_prep_core(b, g, hidden_states, attention_mask, em_cache,
                                  rope_cos, rope_sin, c_attn_w, c_attn_b,
                                  c_proj_w, c_proj_b))

    trace = bool(int(os.environ.get("BASS_KERNEL_TRACE", "0")))
    res = run_bass_kernel_spmd(nc, in_maps, list(range(8)), trace=trace)
    LAST_EXEC_NS = res.exec_time_ns
    LAST_RESULTS = res

    out = np.zeros((B, S, H), dtype=np.float32)
    for core in range(8):
        b = core // 4
        out[b] += res.results[core]["out"]
    out += c_proj_b[None, None, :]
    return out


# revision 12
# speedup vs baseline: 1.0674x; 1.0159x over previous
"""Distributed GQA attention block for TRN2 (8 NeuronCores).

Sharding: core = b*4 + g  (b = batch 0..1, g = kv-head-pair 0..3).
Each core computes qkv for its 8 q-heads / 2 kv-heads, full attention for
those heads, and a partial c_proj ([2048,4096]); host sums the 4 partials
per batch and adds c_proj bias.

All PE-facing tensors are bf16 (fp32 PSUM accumulation). Softmax runs
without max-subtraction (scores are O(30), safe in fp32 exp), and the
additive mask is applied as exp(s+m) = exp(s)*exp(m) with exp(m)
precomputed on host, so ScalarE exps raw PSUM scores directly.

The mask is classified per 128x128 block (skip / identity / general), so
for a causal mask the score+exp+AV work shrinks to the lower-triangular
blocks and the exp(m) multiply runs only on diagonal-crossing blocks.
"""
import sys, os, types

sys.path.insert(0, '/opt/trn_rl_repo')

# Inject the NTFF profile hook module that this image's antenv lacks
# (needed only when tracing; harmless otherwise).
try:
    import antenv
    if "antenv.axon_hooks" not in sys.modules:
        _m = types.ModuleType("antenv.axon_hooks")
        _m._hook = None
        def _set(h, _m=_m): _m._hook = h
        def _get(_m=_m): return _m._hook
        _m.set_axon_ntff_profile_hook = _set
        _m.get_axon_ntff_profile_hook = _get
        sys.modules["antenv.axon_hooks"] = _m
        antenv.axon_hooks = _m
        try:
            from trn_agent_boot.trn_boot import _ntff_profile_via_ctypes
            _set(_ntff_profile_via_ctypes('/opt/axon/libaxon_pjrt.so'))
        except Exception:
            pass
except Exception:
    pass

import numpy as np
import ml_dtypes

import concourse.bass as bass
import concourse.tile as tile
from concourse import bacc, mybir
from concourse.bass_utils import run_bass_kernel_spmd

BF16 = mybir.dt.bfloat16
F32 = mybir.dt.float32
BNP = ml_dtypes.bfloat16

B, S, H = 2, 2048, 4096
NH, NKV, HD = 32, 8, 128
G = NH // NKV                  # 4 q heads per kv head
QH = 8                         # q heads per core
KVH = 2                        # kv heads per core
FT = QH + KVH                  # 10 qk feature tiles per core
ST = S // 128                  # 16 s tiles
KBL = H // 128                 # 32 contraction blocks
SCQ = 256                      # qkv-phase seq chunk
NCQ = S // SCQ                 # 8
QC = 512                       # attention qs chunk
NQC = S // QC                  # 4
NT = QC // 128                 # 4 qs subtiles per chunk
ST2 = ST // 2                  # paired score-tile groups (1024-wide psum)
PC = 512                       # c_proj n chunk
NPC = H // PC                  # 8
SCALE = 1.0 / float(np.sqrt(HD))
VW = HD + 1                    # v-aug row width (ones column for softmax Z)

_CACHE = {}
LAST_EXEC_NS = None
LAST_RESULTS = None


def _build_nc(pattern):
    # pattern[qc][kt][j] classifies the [128 ks x 128 qs] block
    # (ks tile kt, qs subtile j of chunk qc) of exp(mask):
    #   0 => identically zero: block skipped entirely (exact).
    #   1 => identically one: computed, no mask multiply.
    #   2 => general: computed, multiplied by exp(mask).
    nc = bacc.Bacc("TRN2", target_bir_lowering=False, debug=False, num_devices=8)

    xt_e = nc.declare_dram_parameter("xt", [NCQ, 128, KBL * SCQ], BF16, isOutput=False)
    wqk_e = nc.declare_dram_parameter("wqk", [FT, 128, KBL * 128], BF16, isOutput=False)
    wv_e = nc.declare_dram_parameter("wv", [128, KBL * KVH * HD], BF16, isOutput=False)
    bqk_e = nc.declare_dram_parameter("bqk", [128, FT], F32, isOutput=False)
    bv_e = nc.declare_dram_parameter("bv", [128, KVH * HD], F32, isOutput=False)
    cos_e = nc.declare_dram_parameter("cos", [128, S], BF16, isOutput=False)
    sins_e = nc.declare_dram_parameter("sins", [128, S], BF16, isOutput=False)
    em_e = nc.declare_dram_parameter("emask", [NQC, ST2, 128, 2 * QC], BF16,
                                     isOutput=False)
    wp_e = nc.declare_dram_parameter("wp", [NPC, 128, QH * PC], BF16, isOutput=False)
    id_e = nc.declare_dram_parameter("ident", [128, 128], BF16, isOutput=False)
    out_e = nc.declare_dram_parameter("out", [S, H], BF16, isOutput=True)

    ADD = mybir.AluOpType.add
    MUL = mybir.AluOpType.mult
    EXP = mybir.ActivationFunctionType.Exp

    # ---- pattern-derived helpers (all build-time constants) ----
    def blk(qc, kt, j):
        return pattern[qc][kt][j]

    def jmin(qc, kt):
        act = [j for j in range(NT) if blk(qc, kt, j)]
        return act[0] if act else None

    def mrange(qc, kt):
        # qs-subtile range needing the exp(mask) multiply: class-2 blocks,
        # plus class-0 blocks sitting inside the computed range (their
        # exp(mask)=0 zeroes the computed scores).
        jm = jmin(qc, kt)
        if jm is None:
            return None
        need = [j for j in range(jm, NT) if blk(qc, kt, j) != 1]
        if not need:
            return None
        return (need[0], need[-1])

    def act_pairs(qc):
        return [k2 for k2 in range(ST2)
                if jmin(qc, 2 * k2) is not None or jmin(qc, 2 * k2 + 1) is not None]

    def pair_needs_mask(qc, k2):
        return (mrange(qc, 2 * k2) is not None or
                mrange(qc, 2 * k2 + 1) is not None)

    with tile.TileContext(nc) as tc:
        from contextlib import ExitStack
        with ExitStack() as ctx:
            persist = ctx.enter_context(tc.tile_pool(name="persist", bufs=1))

            qkT = [persist.tile([128, S], BF16, tag=f"qkT{i}", name=f"qkT{i}")
                   for i in range(FT)]
            outT = [persist.tile([128, S], BF16, tag=f"outT{h}", name=f"outT{h}")
                    for h in range(QH)]
            vaug = [persist.tile([128, ST * VW], BF16, tag=f"vaug{j}", name=f"vaug{j}")
                    for j in range(KVH)]
            cos_sb = persist.tile([128, S], BF16, tag="cos", name="cos")
            sins_sb = persist.tile([128, S], BF16, tag="sins", name="sins")
            wv_sb = persist.tile([128, KBL * KVH * HD], BF16, tag="wv", name="wv")
            bqk_sb = persist.tile([128, FT], F32, tag="bqk", name="bqk")
            bv_sb = persist.tile([128, KVH * HD], F32, tag="bv", name="bv")
            id_sb = persist.tile([128, 128], BF16, tag="ident", name="ident")

            warm = persist.tile([128, 16], F32, tag="warm", name="warm")
            nc.vector.memset(warm[:], 0.0)
            nc.scalar.activation(warm[:], warm[:],
                                 mybir.ActivationFunctionType.Exp)
            for j in range(KVH):
                nc.vector.memset(vaug[j][:], 1.0)

            # PE HAM warmup: matmuls on a memset tile (no DMA dependency)
            # ride out the ~35us initial weight/activation DMA wait at the
            # cold clock, so the real matmuls start at the full 2.4 GHz
            wmt = persist.tile([128, 128], BF16, tag="wmt", name="wmt")
            nc.vector.memset(wmt[:], 0.25)
            with tc.tile_pool(name="pw", bufs=1, space="PSUM") as pw:
                wps = pw.tile([128, 128], F32, tag="wps", name="wps")
                for _ in range(180):
                    nc.tensor.matmul(wps[:], wmt[:], wmt[:],
                                     start=True, stop=True)

            # ---------------- Phase 1: qkv matmuls + bias + rope ----------------
            # Two f-tile groups: each group's weights load once (no re-reads);
            # k-heads (f=8,9) compute and rope first so attention can begin
            # while the later q-heads still rope.
            def _rope(rp, f):
                rot = rp.tile([128, S], BF16, tag="rot", name="rot")
                nc.sync.dma_start(out=rot[0:64, :], in_=qkT[f][64:128, :])
                nc.sync.dma_start(out=rot[64:128, :], in_=qkT[f][0:64, :])
                t1 = rp.tile([128, S], BF16, tag="t1", name="t1", bufs=1)
                nc.vector.tensor_mul(t1[:], qkT[f][:, :], cos_sb[:, :])
                t2 = rp.tile([128, S], BF16, tag="t2", name="t2", bufs=1)
                nc.vector.tensor_mul(t2[:], rot[:], sins_sb[:, :])
                nc.vector.tensor_add(qkT[f][:, :], t1[:], t2[:])

            rp = ctx.enter_context(tc.tile_pool(name="rope", bufs=2))
            groups = [[8, 9, 0, 1, 2], [3, 4, 5, 6, 7]]
            with tc.tile_pool(name="p1", bufs=2) as p1, \
                 tc.tile_pool(name="ps1", bufs=4, space="PSUM") as ps1:
                for gi, grp in enumerate(groups):
                    # first-needed tiles hit the DMA queues first: w[grp0] and
                    # xt0 split into quarters and interleaved so the first
                    # k-blocks' matmuls can start as early as possible
                    wqs = {}
                    wq_t = p1.tile([128, KBL * 128], BF16, tag="w0",
                                   name="w0", bufs=1)
                    xt0 = p1.tile([128, KBL * SCQ], BF16, tag="xt", name="xt")
                    wq4 = KBL * 128 // 4
                    xq4 = KBL * SCQ // 4
                    for sl in range(4):
                        nc.sync.dma_start(
                            out=wq_t[:, sl * wq4:(sl + 1) * wq4],
                            in_=wqk_e.ap()[grp[0]][:, sl * wq4:(sl + 1) * wq4])
                        nc.sync.dma_start(
                            out=xt0[:, sl * xq4:(sl + 1) * xq4],
                            in_=xt_e.ap()[0][:, sl * xq4:(sl + 1) * xq4])
                    wqs[grp[0]] = wq_t
                    if gi == 1:
                        # needed only from the first rope (early in gi=1)
                        nc.sync.dma_start(out=cos_sb[:], in_=cos_e.ap())
                        nc.sync.dma_start(out=sins_sb[:], in_=sins_e.ap())
                    for i, f in enumerate(grp[1:], start=1):
                        wq_t = p1.tile([128, KBL * 128], BF16, tag=f"w{i}",
                                       name=f"w{i}", bufs=1)
                        nc.sync.dma_start(out=wq_t[:], in_=wqk_e.ap()[f])
                        wqs[f] = wq_t
                        if gi == 0 and i == 1:
                            nc.sync.dma_start(out=bqk_sb[:], in_=bqk_e.ap())
                    if gi == 0:
                        nc.sync.dma_start(out=wv_sb[:], in_=wv_e.ap())
                        nc.sync.dma_start(out=bv_sb[:], in_=bv_e.ap())
                        nc.sync.dma_start(out=id_sb[:], in_=id_e.ap())
                    for c in range(NCQ):
                        if c == 0:
                            xt_t = xt0
                        else:
                            xt_t = p1.tile([128, KBL * SCQ], BF16, tag="xt",
                                           name="xt")
                            nc.sync.dma_start(out=xt_t[:], in_=xt_e.ap()[c])
                        # qk: out[f, s] += wqk[k, f].T @ xT[k, s]
                        for f in grp:
                            psq = ps1.tile([128, SCQ], F32, tag="psq", name="psq")
                            for k in range(KBL):
                                nc.tensor.matmul(
                                    psq[:],
                                    wqs[f][:, k * 128:(k + 1) * 128],
                                    xt_t[:, k * SCQ:(k + 1) * SCQ],
                                    start=(k == 0), stop=(k == KBL - 1))
                            dst = qkT[f][:, c * SCQ:(c + 1) * SCQ]
                            if f < QH:  # fold 1/sqrt(HD) into q
                                nc.vector.tensor_scalar(
                                    dst, psq[:], bqk_sb[:, f:f + 1], SCALE, ADD, MUL)
                            else:
                                nc.vector.tensor_scalar_add(
                                    dst, psq[:], bqk_sb[:, f:f + 1])
                        if gi == 0:
                            # v: out[s, d] += xT[k, s].T @ wv[k, d]
                            for ss in range(SCQ // 128):
                                t_idx = c * (SCQ // 128) + ss
                                psv = ps1.tile([128, KVH * HD], F32, tag="psv",
                                               name="psv")
                                for k in range(KBL):
                                    nc.tensor.matmul(
                                        psv[:],
                                        xt_t[:, k * SCQ + ss * 128:
                                             k * SCQ + ss * 128 + 128],
                                        wv_sb[:, k * KVH * HD:(k + 1) * KVH * HD],
                                        start=(k == 0), stop=(k == KBL - 1))
                                for j in range(KVH):
                                    nc.vector.tensor_add(
                                        vaug[j][:, t_idx * VW: t_idx * VW + HD],
                                        psv[:, j * HD:(j + 1) * HD],
                                        bv_sb[:, j * HD:(j + 1) * HD])
                        # spread group-0's rope across group-1's chunk sweep so
                        # it never head-of-line-blocks the DVE stream
                        if gi == 1 and c < len(groups[0]):
                            _rope(rp, groups[0][c])

            # ---------------- Phase 2+3: attention with c_proj interleave ----
            # c_proj pools open alongside attention: wp weights prefetch on
            # the idle attn DMA window and proj matmul chains are emitted at
            # qc boundaries to fill PE slack while ScalarE exps (the attn
            # bottleneck) keep streaming.
            p3 = ctx.enter_context(tc.tile_pool(name="p3", bufs=3))
            p3o = ctx.enter_context(tc.tile_pool(name="p3o", bufs=3))
            ps3 = ctx.enter_context(tc.tile_pool(name="ps3", bufs=2,
                                                 space="PSUM"))

            def _wp_load(ncj):
                wp_t = p3.tile([128, QH * PC], BF16, tag="wp", name=f"wp{ncj}")
                # 4 slices: the kb-chain can start on slice 0
                w4 = QH * PC // 4
                for sl in range(4):
                    nc.sync.dma_start(
                        out=wp_t[:, sl * w4:(sl + 1) * w4],
                        in_=wp_e.ap()[ncj][:, sl * w4:(sl + 1) * w4])
                return wp_t

            def _proj(ncj, ts, wp_t):
                for t in ts:
                    pcp = ps3.tile([128, PC], F32, tag="cp", name="cp")
                    for kb in range(QH):
                        nc.tensor.matmul(
                            pcp[:],
                            outT[kb][:, t * 128:(t + 1) * 128],
                            wp_t[:, kb * PC:(kb + 1) * PC],
                            start=(kb == 0), stop=(kb == QH - 1))
                    # bf16 partials (summed in f32 on host): halves the
                    # output DMA, which otherwise saturates HBM in the tail
                    osb = p3o.tile([128, PC], BF16, tag="osb", name="osb")
                    nc.vector.tensor_copy(osb[:], pcp[:])
                    nc.sync.dma_start(
                        out=out_e.ap()[t * 128:(t + 1) * 128,
                                       ncj * PC:(ncj + 1) * PC],
                        in_=osb[:])

            with tc.tile_pool(name="p2", bufs=2) as p2, \
                 tc.tile_pool(name="p2n", bufs=4) as p2n, \
                 tc.tile_pool(name="ps_sc", bufs=2, space="PSUM") as ps_sc, \
                 tc.tile_pool(name="ps_av", bufs=2, space="PSUM") as ps_av:
                def _load_mts(qc):
                    mts = {}
                    for k2 in act_pairs(qc):
                        if not pair_needs_mask(qc, k2):
                            continue
                        mt = p2.tile([128, 2 * QC], BF16, tag=f"m{k2}",
                                     name=f"m{k2}", bufs=1)
                        nc.sync.dma_start(out=mt[:], in_=em_e.ap()[qc, k2])
                        mts[k2] = mt
                    return mts

                def _emit_scores(qc, h, mts):
                    kv = h // G
                    p_t = p2.tile([128, ST * QC], BF16, tag="p", name="p")
                    for k2 in act_pairs(qc):
                        psc = ps_sc.tile([128, 2 * QC], F32, tag="sc", name="sc")
                        halves = []
                        for half in range(2):
                            kt = 2 * k2 + half
                            jm = jmin(qc, kt)
                            if jm is None:
                                continue
                            off = jm * 128
                            nc.tensor.matmul(
                                psc[:, half * QC + off:(half + 1) * QC],
                                qkT[QH + kv][:, kt * 128:(kt + 1) * 128],
                                qkT[h][:, qc * QC + off:(qc + 1) * QC],
                                start=True, stop=True)
                            halves.append((half, kt, off))
                        pb = p_t[:, k2 * 2 * QC:(k2 + 1) * 2 * QC]
                        spans = [(half * QC + off, (half + 1) * QC)
                                 for half, kt, off in halves]
                        lo = min(s for s, e in spans)
                        hi = max(e for s, e in spans)
                        useful = sum(e - s for s, e in spans)
                        if (hi - lo) - useful <= 256:
                            # one ACTIVATE (352-cycle fixed cost dominates):
                            # any gap is exp'd garbage that nothing reads
                            nc.scalar.activation(pb[:, lo:hi], psc[:, lo:hi],
                                                 EXP)
                        else:
                            for s, e in spans:
                                nc.scalar.activation(pb[:, s:e], psc[:, s:e],
                                                     EXP)
                        for half, kt, off in halves:
                            mr = mrange(qc, kt)
                            if mr is None:
                                continue
                            ml = half * QC + mr[0] * 128
                            mh = half * QC + (mr[1] + 1) * 128
                            nc.vector.tensor_mul(
                                pb[:, ml:mh], pb[:, ml:mh], mts[k2][:, ml:mh])
                    return p_t

                def _emit_av(qc, h, p_t):
                    kv = h // G
                    # staging holds the 4 normalized [qs,d] subtiles side by
                    # side; one XBAR transpose (out[d, j, q] = stg[q, j*128+d])
                    # moves the whole [QC, HD] block into outT
                    stg = p2n.tile([128, QC], BF16, tag="stg", name="stg")
                    for qs in range(NT):
                        kts = [kt for kt in range(ST) if blk(qc, kt, qs)]
                        pav = ps_av.tile([128, VW], F32, tag="av", name="av")
                        for ki, kt in enumerate(kts):
                            nc.tensor.matmul(
                                pav[:],
                                p_t[:, kt * QC + qs * 128:
                                    kt * QC + qs * 128 + 128],
                                vaug[kv][:, kt * VW:(kt + 1) * VW],
                                start=(ki == 0), stop=(ki == len(kts) - 1))
                        rc = p2n.tile([128, 1], F32, tag="rc", name="rc")
                        nc.vector.reciprocal(rc[:], pav[:, HD:HD + 1])
                        nc.vector.tensor_scalar_mul(
                            stg[:, qs * 128:(qs + 1) * 128],
                            pav[:, 0:HD], rc[:])
                    nc.sync.dma_start_transpose(
                        out=outT[h][:, qc * QC:(qc + 1) * QC].rearrange(
                            "p (j q) -> p j q", j=NT),
                        in_=stg[:])

                # software pipeline: scores(h+1) is emitted before AV(h) so
                # the PE never head-of-line blocks on ScalarE's exp of head h.
                # descending qc: the densest chunk comes first; proj chains
                # for finished qc's t-tiles fill PE slack from then on.
                first = True
                wp_tiles = {}
                pend = [None]

                def _flush():
                    if pend[0] is not None:
                        _emit_av(*pend[0])
                        pend[0] = None

                for qc in range(NQC - 1, -1, -1):
                    mts = _load_mts(qc)
                    if first:
                        # c_proj weight prefetch rides the idle attn DMA window
                        wp_tiles[0] = _wp_load(0)
                        wp_tiles[1] = _wp_load(1)
                    # proj chains over already-completed qc's t-tiles, spread
                    # between heads so they fill PE slack while ScalarE exps
                    if qc == 2:
                        jobs = [(0, t) for t in range(12, 16)]
                    elif qc == 1:
                        jobs = [(0, t) for t in range(8, 12)] + \
                               [(1, t) for t in range(8, 16)]
                    elif qc == 0:
                        jobs = [(0, t) for t in range(4, 8)] + \
                               [(1, t) for t in range(4, 8)] + \
                               [(2, t) for t in range(4, 16)]
                    else:
                        jobs = []
                    ji = 0
                    for h in range(QH):
                        if first and h < len(groups[1]):
                            _rope(rp, groups[1][h])
                        p_t = _emit_scores(qc, h, mts)
                        _flush()
                        pend[0] = (qc, h, p_t)
                        jhi = len(jobs) * (h + 1) // QH
                        while ji < jhi:
                            ncj, t = jobs[ji]
                            _proj(ncj, [t], wp_tiles[ncj])
                            ji += 1
                    first = False
                    _flush()
                    if qc == 1:
                        wp_tiles[2] = _wp_load(2)
                    if qc == 0:
                        _proj(0, range(0, 4), wp_tiles[0])
                        _proj(1, range(0, 4), wp_tiles[1])
                        _proj(2, range(0, 4), wp_tiles[2])
                        for ncj in range(3, NPC):
                            wp_t = _wp_load(ncj)
                            _proj(ncj, range(ST), wp_t)

    nc.compile()
    return nc


def _prep_core(b, g, hidden_states, attention_mask, em_cache,
               rope_cos, rope_sin, c_attn_w, c_attn_b, c_proj_w, c_proj_b):
    x = hidden_states[b]                                   # [S, H] f32
    xt = x.T.astype(BNP)                                   # [H, S]
    # [NCQ, 128, KBL*SCQ]: xt_t[c, p, k*SCQ+j] = xT[k*128+p, c*SCQ+j]
    xt_t = np.ascontiguousarray(
        xt.reshape(KBL, 128, NCQ, SCQ).transpose(2, 1, 0, 3).reshape(
            NCQ, 128, KBL * SCQ))

    # qk weight columns for this core (f-tiles 0..7 = q heads, 8..9 = k heads)
    cols = []
    for h in range(QH):
        j = 2 * g + h // G
        qi = h % G
        c0 = 768 * j + 128 * qi
        cols.append(np.arange(c0, c0 + 128))
    for lkv in range(KVH):
        j = 2 * g + lkv
        c0 = 768 * j + G * HD
        cols.append(np.arange(c0, c0 + 128))
    cols = np.concatenate(cols)                            # [1280]
    wqk = c_attn_w[:, cols].astype(BNP)                    # [H, 1280]
    # [FT, 128, KBL*128]: wqk_t[f, p, k*128+j] = wqk[k*128+p, f*128+j]
    wqk_t = np.ascontiguousarray(
        wqk.reshape(KBL, 128, FT, 128).transpose(2, 1, 0, 3).reshape(
            FT, 128, KBL * 128))
    bqk = np.ascontiguousarray(
        c_attn_b[cols].astype(np.float32).reshape(FT, 128).T)  # [128, FT]

    vcols = np.concatenate([
        np.arange(768 * (2 * g + lkv) + G * HD + HD,
                  768 * (2 * g + lkv) + G * HD + 2 * HD)
        for lkv in range(KVH)])                            # [256]
    wv = c_attn_w[:, vcols].astype(BNP)                    # [H, 256]
    # [128, KBL*256]: wv_t[p, k*256+j] = wv[k*128+p, j]
    wv_t = np.ascontiguousarray(
        wv.reshape(KBL, 128, KVH * HD).transpose(1, 0, 2).reshape(
            128, KBL * KVH * HD))
    bv = np.ascontiguousarray(np.broadcast_to(
        c_attn_b[vcols].astype(np.float32), (128, KVH * HD)))

    cosT = np.ascontiguousarray(rope_cos.T).astype(BNP)    # [128, S]
    sinT = rope_sin.T.copy()
    sinT[0:64, :] *= -1.0
    sinsT = np.ascontiguousarray(sinT).astype(BNP)

    wp = c_proj_w[1024 * g: 1024 * (g + 1), :].astype(BNP)  # [1024, H]
    # [NPC, 128, QH*PC]: wp_t[n, p, kb*PC+j] = wp[kb*128+p, n*PC+j]
    wp_t = np.ascontiguousarray(
        wp.reshape(QH, 128, NPC, PC).transpose(2, 1, 0, 3).reshape(
            NPC, 128, QH * PC))

    ident = np.eye(128, dtype=BNP)

    return {
        "xt": xt_t, "wqk": wqk_t, "wv": wv_t, "bqk": bqk, "bv": bv,
        "cos": cosT, "sins": sinsT, "emask": em_cache[b], "wp": wp_t,
        "ident": ident,
    }


def _emask(attention_mask, b):
    # exp(maskT) tiled [NQC, ST2, 128, 2*QC]:
    # em[qc, k2, p, t*QC+j] = exp(mask[b,0, qc*QC+j, (2*k2+t)*128+p])
    maskT = attention_mask[b, 0].T                         # [S(ks), S(qs)]
    em = np.exp(maskT, dtype=np.float32)
    em_t = np.ascontiguousarray(
        em.reshape(ST2, 2, 128, NQC, QC).transpose(3, 0, 2, 1, 4).reshape(
            NQC, ST2, 128, 2 * QC)).astype(BNP)
    return em_t


def _classify(em_cache):
    # per-block class over ALL batches (one SPMD graph serves every core):
    # 0 = exp(mask) all-zero in every batch, 1 = all-one in every batch,
    # 2 = anything else
    one = np.float32(1.0)
    pat = []
    for qc in range(NQC):
        row = []
        for kt in range(ST):
            k2, half = divmod(kt, 2)
            blocks = []
            for j in range(NT):
                cls = None
                for b in range(B):
                    t = em_cache[b][qc, k2][:, half * QC + j * 128:
                                            half * QC + (j + 1) * 128]
                    if not np.any(t):
                        c = 0
                    elif np.all(t == one):
                        c = 1
                    else:
                        c = 2
                    cls = c if cls is None else (cls if cls == c else 2)
                blocks.append(cls)
            row.append(tuple(blocks))
        pat.append(tuple(row))
    return tuple(pat)


def _rope_np(x, cos, sin):
    h = HD // 2
    x1, x2 = x[..., :h], x[..., h:]
    rot = np.concatenate([-x2, x1], axis=-1)
    return x * cos[None, None] + rot * sin[None, None]


def _kernel_numpy(hidden_states, attention_mask, rope_cos, rope_sin,
                  c_attn_w, c_attn_b, c_proj_w, c_proj_b):
    """Exact reference math in numpy: stability fallback for score regimes
    outside fp32-exp range (never triggers on sanely-scaled inputs)."""
    qkv = hidden_states @ c_attn_w + c_attn_b
    qkv = qkv.reshape(B, S, NKV, G * HD + 2 * HD)
    q = qkv[..., :G * HD].reshape(B, S, NH, HD).transpose(0, 2, 1, 3)
    k = qkv[..., G * HD:G * HD + HD].transpose(0, 2, 1, 3)
    v = qkv[..., G * HD + HD:].transpose(0, 2, 1, 3)
    q = _rope_np(q, rope_cos, rope_sin)
    k = _rope_np(k, rope_cos, rope_sin)
    k = np.repeat(k, G, axis=1)
    v = np.repeat(v, G, axis=1)
    out = np.empty((B, NH, S, HD), np.float32)
    for b in range(B):
        for h in range(NH):
            s_ = (q[b, h] @ k[b, h].T) * SCALE + attention_mask[b, 0]
            s_ = s_ - s_.max(axis=-1, keepdims=True)
            p = np.exp(s_, dtype=np.float32)
            out[b, h] = (p / p.sum(axis=-1, keepdims=True)) @ v[b, h]
    out = out.transpose(0, 2, 1, 3).reshape(B, S, H)
    return out @ c_proj_w + c_proj_b


def _score_scale_probe(hidden_states, attention_mask, rope_cos, rope_sin,
                       c_attn_w, c_attn_b):
    """Upper estimate of max |score + mask| via a small exact sample."""
    x = hidden_states[0, :256]                      # [256, H]
    j = 0
    qc = c_attn_w[:, 768 * j:768 * j + 128]
    kc = c_attn_w[:, 768 * j + 512:768 * j + 640]
    q = (x[:32] @ qc + c_attn_b[768 * j:768 * j + 128])[None, None]
    k = (x @ kc + c_attn_b[768 * j + 512:768 * j + 640])[None, None]
    q = _rope_np(q, rope_cos[:32], rope_sin[:32])[0, 0]
    k = _rope_np(k, rope_cos[:256], rope_sin[:256])[0, 0]
    s_ = (q @ k.T) * SCALE
    m = attention_mask[0, 0, :32, :256]
    pos = np.abs(s_).std() * 8.0 + max(0.0, float(m.max()))
    return pos


def kernel(hidden_states, attention_mask, rope_cos, rope_sin,
           c_attn_w, c_attn_b, c_proj_w, c_proj_b):
    global LAST_EXEC_NS, LAST_RESULTS
    hidden_states = np.asarray(hidden_states, dtype=np.float32)
    attention_mask = np.asarray(attention_mask, dtype=np.float32)
    rope_cos = np.asarray(rope_cos, dtype=np.float32)
    rope_sin = np.asarray(rope_sin, dtype=np.float32)
    c_attn_w = np.asarray(c_attn_w, dtype=np.float32)
    c_attn_b = np.asarray(c_attn_b, dtype=np.float32)
    c_proj_w = np.asarray(c_proj_w, dtype=np.float32)
    c_proj_b = np.asarray(c_proj_b, dtype=np.float32)

    if _score_scale_probe(hidden_states, attention_mask, rope_cos,
                          rope_sin, c_attn_w, c_attn_b) > 75.0:
        # scores would overflow fp32 exp without per-row max subtraction;
        # use the exact (slow) host path rather than returning garbage
        LAST_EXEC_NS = None
        return _kernel_numpy(hidden_states, attention_mask, rope_cos,
                             rope_sin, c_attn_w, c_attn_b, c_proj_w,
                             c_proj_b)

    em_cache = [_emask(attention_mask, b) for b in range(B)]
    pattern = _classify(em_cache)
    # safety: every (qc, qs-subtile) needs at least one contributing ks
    # block, else softmax Z would be empty -> fall back to dense pattern
    degenerate = any(
        not any(pattern[qc][kt][j] for kt in range(ST))
        for qc in range(NQC) for j in range(NT))
    if degenerate:
        pattern = tuple(
            tuple(tuple(2 for _ in range(NT)) for _ in range(ST))
            for _ in range(NQC))

    if pattern not in _CACHE:
        _CACHE[pattern] = _build_nc(pattern)
    nc = _CACHE[pattern]
    in_maps = []
    for core in range(8):
        b, g = divmod(core, 4)
        in_maps.append(_prep_core(b, g, hidden_states, attention_mask, em_cache,
                                  rope_cos, rope_sin, c_attn_w, c_attn_b,
                                  c_proj_w, c_proj_b))

    trace = bool(int(os.environ.get("BASS_KERNEL_TRACE", "0")))
    res = run_bass_kernel_spmd(nc, in_maps, list(range(8)), trace=trace)
    LAST_EXEC_NS = res.exec_time_ns
    LAST_RESULTS = res

    out = np.zeros((B, S, H), dtype=np.float32)
    for core in range(8):
        b = core // 4
        out[b] += res.results[core]["out"]
    out += c_proj_b[None, None, :]
    return out


# revision 22
# speedup vs baseline: 1.0713x; 1.0037x over previous
"""Distributed GQA attention block for TRN2 (8 NeuronCores).

Sharding: core = b*4 + g  (b = batch 0..1, g = kv-head-pair 0..3).
Each core computes qkv for its 8 q-heads / 2 kv-heads, full attention for
those heads, and a partial c_proj ([2048,4096]); host sums the 4 partials
per batch and adds c_proj bias.

All PE-facing tensors are bf16 (fp32 PSUM accumulation). Softmax runs
without max-subtraction (scores are O(30), safe in fp32 exp), and the
additive mask is applied as exp(s+m) = exp(s)*exp(m) with exp(m)
precomputed on host, so ScalarE exps raw PSUM scores directly.

The mask is classified per 128x128 block (skip / identity / general), so
for a causal mask the score+exp+AV work shrinks to the lower-triangular
blocks and the exp(m) multiply runs only on diagonal-crossing blocks.
"""
import sys, os, types

sys.path.insert(0, '/opt/trn_rl_repo')

# Inject the NTFF profile hook module that this image's antenv lacks
# (needed only when tracing; harmless otherwise).
try:
    import antenv
    if "antenv.axon_hooks" not in sys.modules:
        _m = types.ModuleType("antenv.axon_hooks")
        _m._hook = None
        def _set(h, _m=_m): _m._hook = h
        def _get(_m=_m): return _m._hook
        _m.set_axon_ntff_profile_hook = _set
        _m.get_axon_ntff_profile_hook = _get
        sys.modules["antenv.axon_hooks"] = _m
        antenv.axon_hooks = _m
        try:
            from trn_agent_boot.trn_boot import _ntff_profile_via_ctypes
            _set(_ntff_profile_via_ctypes('/opt/axon/libaxon_pjrt.so'))
        except Exception:
            pass
except Exception:
    pass

import numpy as np
import ml_dtypes

import concourse.bass as bass
import concourse.tile as tile
from concourse import bacc, mybir
from concourse.bass_utils import run_bass_kernel_spmd

BF16 = mybir.dt.bfloat16
F32 = mybir.dt.float32
BNP = ml_dtypes.bfloat16

B, S, H = 2, 2048, 4096
NH, NKV, HD = 32, 8, 128
G = NH // NKV                  # 4 q heads per kv head
QH = 8                         # q heads per core
KVH = 2                        # kv heads per core
FT = QH + KVH                  # 10 qk feature tiles per core
ST = S // 128                  # 16 s tiles
KBL = H // 128                 # 32 contraction blocks
SCQ = 256                      # qkv-phase seq chunk
NCQ = S // SCQ                 # 8
QC = 512                       # attention qs chunk
NQC = S // QC                  # 4
NT = QC // 128                 # 4 qs subtiles per chunk
ST2 = ST // 2                  # paired score-tile groups (1024-wide psum)
PC = 512                       # c_proj n chunk
NPC = H // PC                  # 8
SCALE = 1.0 / float(np.sqrt(HD))
VW = HD + 1                    # v-aug row width (ones column for softmax Z)

_CACHE = {}
LAST_EXEC_NS = None
LAST_RESULTS = None


def _build_nc(pattern):
    # pattern[qc][kt][j] classifies the [128 ks x 128 qs] block
    # (ks tile kt, qs subtile j of chunk qc) of exp(mask):
    #   0 => identically zero: block skipped entirely (exact).
    #   1 => identically one: computed, no mask multiply.
    #   2 => general: computed, multiplied by exp(mask).
    nc = bacc.Bacc("TRN2", target_bir_lowering=False, debug=False, num_devices=8)

    xt_e = nc.declare_dram_parameter("xt", [NCQ, 128, KBL * SCQ], BF16, isOutput=False)
    wqk_e = nc.declare_dram_parameter("wqk", [FT, 128, KBL * 128], BF16, isOutput=False)
    wv_e = nc.declare_dram_parameter("wv", [128, KBL * KVH * HD], BF16, isOutput=False)
    bqk_e = nc.declare_dram_parameter("bqk", [128, FT], F32, isOutput=False)
    bv_e = nc.declare_dram_parameter("bv", [128, KVH * HD], F32, isOutput=False)
    cos_e = nc.declare_dram_parameter("cos", [128, S], BF16, isOutput=False)
    sins_e = nc.declare_dram_parameter("sins", [128, S], BF16, isOutput=False)
    em_e = nc.declare_dram_parameter("emask", [NQC, ST2, 128, 2 * QC], BF16,
                                     isOutput=False)
    wp_e = nc.declare_dram_parameter("wp", [NPC, 128, QH * PC], BF16, isOutput=False)
    id_e = nc.declare_dram_parameter("ident", [128, 128], BF16, isOutput=False)
    out_e = nc.declare_dram_parameter("out", [S, H], BF16, isOutput=True)

    ADD = mybir.AluOpType.add
    MUL = mybir.AluOpType.mult
    EXP = mybir.ActivationFunctionType.Exp

    # ---- pattern-derived helpers (all build-time constants) ----
    def blk(qc, kt, j):
        return pattern[qc][kt][j]

    def jmin(qc, kt):
        act = [j for j in range(NT) if blk(qc, kt, j)]
        return act[0] if act else None

    def mrange(qc, kt):
        # qs-subtile range needing the exp(mask) multiply: class-2 blocks,
        # plus class-0 blocks sitting inside the computed range (their
        # exp(mask)=0 zeroes the computed scores).
        jm = jmin(qc, kt)
        if jm is None:
            return None
        need = [j for j in range(jm, NT) if blk(qc, kt, j) != 1]
        if not need:
            return None
        return (need[0], need[-1])

    def act_pairs(qc):
        return [k2 for k2 in range(ST2)
                if jmin(qc, 2 * k2) is not None or jmin(qc, 2 * k2 + 1) is not None]

    # SBUF shape knobs: with few masked pairs per qc (causal), the freed
    # space funds a deep c_proj weight ring; dense masks get a shallow one
    max_mask_pairs = max(
        sum(1 for k2 in range(ST2)
            if mrange(qc, 2 * k2) is not None or mrange(qc, 2 * k2 + 1) is not None)
        for qc in range(NQC))
    mts_bufs = 2 if max_mask_pairs <= 3 else 1
    wp_bufs = 6 if max_mask_pairs <= 3 else 3

    def pair_needs_mask(qc, k2):
        return (mrange(qc, 2 * k2) is not None or
                mrange(qc, 2 * k2 + 1) is not None)

    with tile.TileContext(nc) as tc:
        from contextlib import ExitStack
        with ExitStack() as ctx:
            persist = ctx.enter_context(tc.tile_pool(name="persist", bufs=1))

            qkT = [persist.tile([128, S], BF16, tag=f"qkT{i}", name=f"qkT{i}")
                   for i in range(FT)]
            outT = [persist.tile([128, S], BF16, tag=f"outT{h}", name=f"outT{h}")
                    for h in range(QH)]
            vaug = [persist.tile([128, ST * VW], BF16, tag=f"vaug{j}", name=f"vaug{j}")
                    for j in range(KVH)]
            cos_sb = persist.tile([128, S], BF16, tag="cos", name="cos")
            sins_sb = persist.tile([128, S], BF16, tag="sins", name="sins")
            wv_sb = persist.tile([128, KBL * KVH * HD], BF16, tag="wv", name="wv")
            bqk_sb = persist.tile([128, FT], F32, tag="bqk", name="bqk")
            bv_sb = persist.tile([128, KVH * HD], F32, tag="bv", name="bv")
            id_sb = persist.tile([128, 128], BF16, tag="ident", name="ident")

            warm = persist.tile([128, 16], F32, tag="warm", name="warm")
            nc.vector.memset(warm[:], 0.0)
            nc.scalar.activation(warm[:], warm[:],
                                 mybir.ActivationFunctionType.Exp)
            for j in range(KVH):
                nc.vector.memset(vaug[j][:], 1.0)

            # PE HAM warmup: matmuls on a memset tile (no DMA dependency)
            # ride out the ~35us initial weight/activation DMA wait at the
            # cold clock, so the real matmuls start at the full 2.4 GHz
            wmt = persist.tile([128, 128], BF16, tag="wmt", name="wmt")
            nc.vector.memset(wmt[:], 0.25)
            with tc.tile_pool(name="pw", bufs=1, space="PSUM") as pw:
                wps = pw.tile([128, 128], F32, tag="wps", name="wps")
                for _ in range(180):
                    nc.tensor.matmul(wps[:], wmt[:], wmt[:],
                                     start=True, stop=True)

            # ---------------- Phase 1: qkv matmuls + bias + rope ----------------
            # Two f-tile groups: each group's weights load once (no re-reads);
            # k-heads (f=8,9) compute and rope first so attention can begin
            # while the later q-heads still rope.
            def _rope(rp, f):
                rot = rp.tile([128, S], BF16, tag="rot", name="rot")
                nc.sync.dma_start(out=rot[0:64, :], in_=qkT[f][64:128, :])
                nc.sync.dma_start(out=rot[64:128, :], in_=qkT[f][0:64, :])
                t1 = rp.tile([128, S], BF16, tag="t1", name="t1", bufs=1)
                nc.vector.tensor_mul(t1[:], qkT[f][:, :], cos_sb[:, :])
                t2 = rp.tile([128, S], BF16, tag="t2", name="t2", bufs=1)
                nc.vector.tensor_mul(t2[:], rot[:], sins_sb[:, :])
                nc.vector.tensor_add(qkT[f][:, :], t1[:], t2[:])

            groups = [[8, 9, 0, 1, 2], [3, 4, 5, 6, 7]]
            with tc.tile_pool(name="rope", bufs=2) as rp, \
                 tc.tile_pool(name="p1", bufs=2) as p1, \
                 tc.tile_pool(name="ps1", bufs=4, space="PSUM") as ps1:
                for gi, grp in enumerate(groups):
                    # first-needed tiles hit the DMA queues first: w[grp0] and
                    # xt0 split into quarters and interleaved so the first
                    # k-blocks' matmuls can start as early as possible
                    wqs = {}
                    wq_t = p1.tile([128, KBL * 128], BF16, tag="w0",
                                   name="w0", bufs=1)
                    xt0 = p1.tile([128, KBL * SCQ], BF16, tag="xt", name="xt")
                    wq4 = KBL * 128 // 4
                    xq4 = KBL * SCQ // 4
                    for sl in range(4):
                        nc.sync.dma_start(
                            out=wq_t[:, sl * wq4:(sl + 1) * wq4],
                            in_=wqk_e.ap()[grp[0]][:, sl * wq4:(sl + 1) * wq4])
                        nc.sync.dma_start(
                            out=xt0[:, sl * xq4:(sl + 1) * xq4],
                            in_=xt_e.ap()[0][:, sl * xq4:(sl + 1) * xq4])
                    wqs[grp[0]] = wq_t
                    if gi == 1:
                        # needed only from the first rope (early in gi=1)
                        nc.sync.dma_start(out=cos_sb[:], in_=cos_e.ap())
                        nc.sync.dma_start(out=sins_sb[:], in_=sins_e.ap())
                    for i, f in enumerate(grp[1:], start=1):
                        wq_t = p1.tile([128, KBL * 128], BF16, tag=f"w{i}",
                                       name=f"w{i}", bufs=1)
                        nc.sync.dma_start(out=wq_t[:], in_=wqk_e.ap()[f])
                        wqs[f] = wq_t
                        if gi == 0 and i == 1:
                            nc.sync.dma_start(out=bqk_sb[:], in_=bqk_e.ap())
                    if gi == 0:
                        nc.sync.dma_start(out=wv_sb[:], in_=wv_e.ap())
                        nc.sync.dma_start(out=bv_sb[:], in_=bv_e.ap())
                        nc.sync.dma_start(out=id_sb[:], in_=id_e.ap())
                    for c in range(NCQ):
                        if c == 0:
                            xt_t = xt0
                        else:
                            xt_t = p1.tile([128, KBL * SCQ], BF16, tag="xt",
                                           name="xt")
                            nc.sync.dma_start(out=xt_t[:], in_=xt_e.ap()[c])
                        # qk: out[f, s] += wqk[k, f].T @ xT[k, s]
                        for f in grp:
                            psq = ps1.tile([128, SCQ], F32, tag="psq", name="psq")
                            for k in range(KBL):
                                nc.tensor.matmul(
                                    psq[:],
                                    wqs[f][:, k * 128:(k + 1) * 128],
                                    xt_t[:, k * SCQ:(k + 1) * SCQ],
                                    start=(k == 0), stop=(k == KBL - 1))
                            dst = qkT[f][:, c * SCQ:(c + 1) * SCQ]
                            if f < QH:  # fold 1/sqrt(HD) into q
                                nc.vector.tensor_scalar(
                                    dst, psq[:], bqk_sb[:, f:f + 1], SCALE, ADD, MUL)
                            else:
                                nc.vector.tensor_scalar_add(
                                    dst, psq[:], bqk_sb[:, f:f + 1])
                        if gi == 0:
                            # v: out[s, d] += xT[k, s].T @ wv[k, d]
                            for ss in range(SCQ // 128):
                                t_idx = c * (SCQ // 128) + ss
                                psv = ps1.tile([128, KVH * HD], F32, tag="psv",
                                               name="psv")
                                for k in range(KBL):
                                    nc.tensor.matmul(
                                        psv[:],
                                        xt_t[:, k * SCQ + ss * 128:
                                             k * SCQ + ss * 128 + 128],
                                        wv_sb[:, k * KVH * HD:(k + 1) * KVH * HD],
                                        start=(k == 0), stop=(k == KBL - 1))
                                for j in range(KVH):
                                    nc.vector.tensor_add(
                                        vaug[j][:, t_idx * VW: t_idx * VW + HD],
                                        psv[:, j * HD:(j + 1) * HD],
                                        bv_sb[:, j * HD:(j + 1) * HD])
                        # spread group-0's rope across group-1's chunk sweep so
                        # it never head-of-line-blocks the DVE stream
                        if gi == 1 and c < len(groups[0]):
                            _rope(rp, groups[0][c])
                # group-1 ropes at phase-1 end: DVE is idle here and the
                # first attn heads that need them (h>=3) run ~20us later
                for f in groups[1]:
                    _rope(rp, f)

            # ---------------- Phase 2+3: attention with c_proj interleave ----
            # c_proj pools open alongside attention: wp weights prefetch on
            # the idle attn DMA window and proj matmul chains are emitted at
            # qc boundaries to fill PE slack while ScalarE exps (the attn
            # bottleneck) keep streaming.
            p3 = ctx.enter_context(tc.tile_pool(name="p3", bufs=wp_bufs))
            p3o = ctx.enter_context(tc.tile_pool(name="p3o", bufs=3))
            ps3 = ctx.enter_context(tc.tile_pool(name="ps3", bufs=2,
                                                 space="PSUM"))

            def _wp_load(ncj):
                wp_t = p3.tile([128, QH * PC], BF16, tag="wp", name=f"wp{ncj}")
                # 4 slices: the kb-chain can start on slice 0
                w4 = QH * PC // 4
                for sl in range(4):
                    nc.sync.dma_start(
                        out=wp_t[:, sl * w4:(sl + 1) * w4],
                        in_=wp_e.ap()[ncj][:, sl * w4:(sl + 1) * w4])
                return wp_t

            def _proj(ncj, ts, wp_t):
                for t in ts:
                    pcp = ps3.tile([128, PC], F32, tag="cp", name="cp")
                    for kb in range(QH):
                        nc.tensor.matmul(
                            pcp[:],
                            outT[kb][:, t * 128:(t + 1) * 128],
                            wp_t[:, kb * PC:(kb + 1) * PC],
                            start=(kb == 0), stop=(kb == QH - 1))
                    # bf16 partials (summed in f32 on host): halves the
                    # output DMA, which otherwise saturates HBM in the tail
                    osb = p3o.tile([128, PC], BF16, tag="osb", name="osb")
                    nc.vector.tensor_copy(osb[:], pcp[:])
                    nc.sync.dma_start(
                        out=out_e.ap()[t * 128:(t + 1) * 128,
                                       ncj * PC:(ncj + 1) * PC],
                        in_=osb[:])

            with tc.tile_pool(name="p2", bufs=2) as p2, \
                 tc.tile_pool(name="p2n", bufs=4) as p2n, \
                 tc.tile_pool(name="ps_sc", bufs=2, space="PSUM") as ps_sc, \
                 tc.tile_pool(name="ps_av", bufs=2, space="PSUM") as ps_av:
                def _load_mts(qc):
                    mts = {}
                    idx = 0
                    for k2 in act_pairs(qc):
                        if not pair_needs_mask(qc, k2):
                            continue
                        # tag by within-qc index: distinct tags for all
                        # simultaneously-live tiles, ring depth 2 so the next
                        # qc's loads prefetch behind the current qc's reads
                        mt = p2.tile([128, 2 * QC], BF16, tag=f"m{idx}",
                                     name=f"m{qc}_{k2}", bufs=mts_bufs)
                        nc.sync.dma_start(out=mt[:], in_=em_e.ap()[qc, k2])
                        mts[k2] = mt
                        idx += 1
                    return mts

                def _emit_scores(qc, h, mts):
                    kv = h // G
                    p_t = p2.tile([128, ST * QC], BF16, tag="p", name="p")
                    for k2 in act_pairs(qc):
                        psc = ps_sc.tile([128, 2 * QC], F32, tag="sc", name="sc")
                        halves = []
                        for half in range(2):
                            kt = 2 * k2 + half
                            jm = jmin(qc, kt)
                            if jm is None:
                                continue
                            off = jm * 128
                            nc.tensor.matmul(
                                psc[:, half * QC + off:(half + 1) * QC],
                                qkT[QH + kv][:, kt * 128:(kt + 1) * 128],
                                qkT[h][:, qc * QC + off:(qc + 1) * QC],
                                start=True, stop=True)
                            halves.append((half, kt, off))
                        pb = p_t[:, k2 * 2 * QC:(k2 + 1) * 2 * QC]
                        spans = [(half * QC + off, (half + 1) * QC)
                                 for half, kt, off in halves]
                        lo = min(s for s, e in spans)
                        hi = max(e for s, e in spans)
                        useful = sum(e - s for s, e in spans)
                        if (hi - lo) - useful <= 256:
                            # one ACTIVATE (352-cycle fixed cost dominates):
                            # any gap is exp'd garbage that nothing reads
                            nc.scalar.activation(pb[:, lo:hi], psc[:, lo:hi],
                                                 EXP)
                        else:
                            for s, e in spans:
                                nc.scalar.activation(pb[:, s:e], psc[:, s:e],
                                                     EXP)
                        for half, kt, off in halves:
                            mr = mrange(qc, kt)
                            if mr is None:
                                continue
                            ml = half * QC + mr[0] * 128
                            mh = half * QC + (mr[1] + 1) * 128
                            nc.vector.tensor_mul(
                                pb[:, ml:mh], pb[:, ml:mh], mts[k2][:, ml:mh])
                    return p_t

                def _emit_av(qc, h, p_t):
                    kv = h // G
                    # staging holds the 4 normalized [qs,d] subtiles side by
                    # side; one XBAR transpose (out[d, j, q] = stg[q, j*128+d])
                    # moves the whole [QC, HD] block into outT
                    stg = p2n.tile([128, QC], BF16, tag="stg", name="stg")
                    for qs in range(NT):
                        kts = [kt for kt in range(ST) if blk(qc, kt, qs)]
                        pav = ps_av.tile([128, VW], F32, tag="av", name="av")
                        for ki, kt in enumerate(kts):
                            nc.tensor.matmul(
                                pav[:],
                                p_t[:, kt * QC + qs * 128:
                                    kt * QC + qs * 128 + 128],
                                vaug[kv][:, kt * VW:(kt + 1) * VW],
                                start=(ki == 0), stop=(ki == len(kts) - 1))
                        rc = p2n.tile([128, 1], F32, tag="rc", name="rc")
                        nc.vector.reciprocal(rc[:], pav[:, HD:HD + 1])
                        nc.vector.tensor_scalar_mul(
                            stg[:, qs * 128:(qs + 1) * 128],
                            pav[:, 0:HD], rc[:])
                    nc.sync.dma_start_transpose(
                        out=outT[h][:, qc * QC:(qc + 1) * QC].rearrange(
                            "p (j q) -> p j q", j=NT),
                        in_=stg[:])

                # software pipeline: scores(h+1) is emitted before AV(h) so
                # the PE never head-of-line blocks on ScalarE's exp of head h.
                # descending qc: the densest chunk comes first; proj chains
                # for finished qc's t-tiles fill PE slack from then on.
                first = True
                wp_tiles = {}
                pend = [None]

                def _flush():
                    if pend[0] is not None:
                        _emit_av(*pend[0])
                        pend[0] = None

                for qc in range(NQC - 1, -1, -1):
                    mts = _load_mts(qc)
                    if first:
                        # c_proj weight prefetch rides the idle attn DMA window
                        wp_tiles[0] = _wp_load(0)
                        wp_tiles[1] = _wp_load(1)
                    # proj chains over already-completed qc's t-tiles, spread
                    # between heads so they fill PE slack while ScalarE exps
                    if wp_bufs >= 6:
                        if qc == 2:
                            jobs = [(n, t) for n in (0, 1)
                                    for t in range(12, 16)]
                        elif qc == 1:
                            jobs = [(n, t) for n in (0, 1)
                                    for t in range(8, 12)] \
                                + [(n, t) for n in (2, 3)
                                   for t in range(8, 16)]
                        elif qc == 0:
                            jobs = [(n, t) for n in range(4)
                                    for t in range(4, 8)] \
                                + [(n, t) for n in (4, 5)
                                   for t in range(4, 16)]
                        else:
                            jobs = []
                    else:
                        if qc == 2:
                            jobs = [(0, t) for t in range(12, 16)]
                        elif qc == 1:
                            jobs = [(0, t) for t in range(8, 12)] + \
                                   [(1, t) for t in range(8, 16)]
                        elif qc == 0:
                            jobs = [(0, t) for t in range(4, 8)] + \
                                   [(1, t) for t in range(4, 8)] + \
                                   [(2, t) for t in range(4, 16)]
                        else:
                            jobs = []
                    ji = 0
                    for h in range(QH):
                        p_t = _emit_scores(qc, h, mts)
                        _flush()
                        pend[0] = (qc, h, p_t)
                        jhi = len(jobs) * (h + 1) // QH
                        while ji < jhi:
                            ncj, t = jobs[ji]
                            _proj(ncj, [t], wp_tiles[ncj])
                            ji += 1
                    first = False
                    _flush()
                    if wp_bufs >= 6:
                        if qc == 3:
                            wp_tiles[2] = _wp_load(2)
                            wp_tiles[3] = _wp_load(3)
                        elif qc == 2:
                            wp_tiles[4] = _wp_load(4)
                            wp_tiles[5] = _wp_load(5)
                        elif qc == 0:
                            # t0-3 stubs free ring slots; late wp loads are
                            # emitted as soon as a slot's reads are all queued
                            _proj(0, range(0, 4), wp_tiles[0])
                            wp_tiles[6] = _wp_load(6)
                            _proj(1, range(0, 4), wp_tiles[1])
                            wp_tiles[7] = _wp_load(7)
                            for ncj in range(2, 6):
                                _proj(ncj, range(0, 4), wp_tiles[ncj])
                            _proj(6, range(ST), wp_tiles[6])
                            _proj(7, range(ST), wp_tiles[7])
                    else:
                        if qc == 1:
                            wp_tiles[2] = _wp_load(2)
                        elif qc == 0:
                            _proj(0, range(0, 4), wp_tiles[0])
                            _proj(1, range(0, 4), wp_tiles[1])
                            _proj(2, range(0, 4), wp_tiles[2])
                            for ncj in range(3, NPC):
                                wp_t = _wp_load(ncj)
                                _proj(ncj, range(ST), wp_t)

    nc.compile()
    return nc


def _prep_core(b, g, hidden_states, attention_mask, em_cache,
               rope_cos, rope_sin, c_attn_w, c_attn_b, c_proj_w, c_proj_b):
    x = hidden_states[b]                                   # [S, H] f32
    xt = x.T.astype(BNP)                                   # [H, S]
    # [NCQ, 128, KBL*SCQ]: xt_t[c, p, k*SCQ+j] = xT[k*128+p, c*SCQ+j]
    xt_t = np.ascontiguousarray(
        xt.reshape(KBL, 128, NCQ, SCQ).transpose(2, 1, 0, 3).reshape(
            NCQ, 128, KBL * SCQ))

    # qk weight columns for this core (f-tiles 0..7 = q heads, 8..9 = k heads)
    cols = []
    for h in range(QH):
        j = 2 * g + h // G
        qi = h % G
        c0 = 768 * j + 128 * qi
        cols.append(np.arange(c0, c0 + 128))
    for lkv in range(KVH):
        j = 2 * g + lkv
        c0 = 768 * j + G * HD
        cols.append(np.arange(c0, c0 + 128))
    cols = np.concatenate(cols)                            # [1280]
    wqk = c_attn_w[:, cols].astype(BNP)                    # [H, 1280]
    # [FT, 128, KBL*128]: wqk_t[f, p, k*128+j] = wqk[k*128+p, f*128+j]
    wqk_t = np.ascontiguousarray(
        wqk.reshape(KBL, 128, FT, 128).transpose(2, 1, 0, 3).reshape(
            FT, 128, KBL * 128))
    bqk = np.ascontiguousarray(
        c_attn_b[cols].astype(np.float32).reshape(FT, 128).T)  # [128, FT]

    vcols = np.concatenate([
        np.arange(768 * (2 * g + lkv) + G * HD + HD,
                  768 * (2 * g + lkv) + G * HD + 2 * HD)
        for lkv in range(KVH)])                            # [256]
    wv = c_attn_w[:, vcols].astype(BNP)                    # [H, 256]
    # [128, KBL*256]: wv_t[p, k*256+j] = wv[k*128+p, j]
    wv_t = np.ascontiguousarray(
        wv.reshape(KBL, 128, KVH * HD).transpose(1, 0, 2).reshape(
            128, KBL * KVH * HD))
    bv = np.ascontiguousarray(np.broadcast_to(
        c_attn_b[vcols].astype(np.float32), (128, KVH * HD)))

    cosT = np.ascontiguousarray(rope_cos.T).astype(BNP)    # [128, S]
    sinT = rope_sin.T.copy()
    sinT[0:64, :] *= -1.0
    sinsT = np.ascontiguousarray(sinT).astype(BNP)

    wp = c_proj_w[1024 * g: 1024 * (g + 1), :].astype(BNP)  # [1024, H]
    # [NPC, 128, QH*PC]: wp_t[n, p, kb*PC+j] = wp[kb*128+p, n*PC+j]
    wp_t = np.ascontiguousarray(
        wp.reshape(QH, 128, NPC, PC).transpose(2, 1, 0, 3).reshape(
            NPC, 128, QH * PC))

    ident = np.eye(128, dtype=BNP)

    return {
        "xt": xt_t, "wqk": wqk_t, "wv": wv_t, "bqk": bqk, "bv": bv,
        "cos": cosT, "sins": sinsT, "emask": em_cache[b], "wp": wp_t,
        "ident": ident,
    }


def _emask(attention_mask, b):
    # exp(maskT) tiled [NQC, ST2, 128, 2*QC]:
    # em[qc, k2, p, t*QC+j] = exp(mask[b,0, qc*QC+j, (2*k2+t)*128+p])
    maskT = attention_mask[b, 0].T                         # [S(ks), S(qs)]
    em = np.exp(maskT, dtype=np.float32)
    em_t = np.ascontiguousarray(
        em.reshape(ST2, 2, 128, NQC, QC).transpose(3, 0, 2, 1, 4).reshape(
            NQC, ST2, 128, 2 * QC)).astype(BNP)
    return em_t


def _classify(em_cache):
    # per-block class over ALL batches (one SPMD graph serves every core):
    # 0 = exp(mask) all-zero in every batch, 1 = all-one in every batch,
    # 2 = anything else
    one = np.float32(1.0)
    pat = []
    for qc in range(NQC):
        row = []
        for kt in range(ST):
            k2, half = divmod(kt, 2)
            blocks = []
            for j in range(NT):
                cls = None
                for b in range(B):
                    t = em_cache[b][qc, k2][:, half * QC + j * 128:
                                            half * QC + (j + 1) * 128]
                    if not np.any(t):
                        c = 0
                    elif np.all(t == one):
                        c = 1
                    else:
                        c = 2
                    cls = c if cls is None else (cls if cls == c else 2)
                blocks.append(cls)
            row.append(tuple(blocks))
        pat.append(tuple(row))
    return tuple(pat)


def _rope_np(x, cos, sin):
    h = HD // 2
    x1, x2 = x[..., :h], x[..., h:]
    rot = np.concatenate([-x2, x1], axis=-1)
    return x * cos[None, None] + rot * sin[None, None]


def _kernel_numpy(hidden_states, attention_mask, rope_cos, rope_sin,
                  c_attn_w, c_attn_b, c_proj_w, c_proj_b):
    """Exact reference math in numpy: stability fallback for score regimes
    outside fp32-exp range (never triggers on sanely-scaled inputs)."""
    qkv = hidden_states @ c_attn_w + c_attn_b
    qkv = qkv.reshape(B, S, NKV, G * HD + 2 * HD)
    q = qkv[..., :G * HD].reshape(B, S, NH, HD).transpose(0, 2, 1, 3)
    k = qkv[..., G * HD:G * HD + HD].transpose(0, 2, 1, 3)
    v = qkv[..., G * HD + HD:].transpose(0, 2, 1, 3)
    q = _rope_np(q, rope_cos, rope_sin)
    k = _rope_np(k, rope_cos, rope_sin)
    k = np.repeat(k, G, axis=1)
    v = np.repeat(v, G, axis=1)
    out = np.empty((B, NH, S, HD), np.float32)
    for b in range(B):
        for h in range(NH):
            s_ = (q[b, h] @ k[b, h].T) * SCALE + attention_mask[b, 0]
            s_ = s_ - s_.max(axis=-1, keepdims=True)
            p = np.exp(s_, dtype=np.float32)
            out[b, h] = (p / p.sum(axis=-1, keepdims=True)) @ v[b, h]
    out = out.transpose(0, 2, 1, 3).reshape(B, S, H)
    return out @ c_proj_w + c_proj_b


def _score_scale_probe(hidden_states, attention_mask, rope_cos, rope_sin,
                       c_attn_w, c_attn_b):
    """Upper estimate of max |score + mask| via a small exact sample."""
    x = hidden_states[0, :256]                      # [256, H]
    j = 0
    qc = c_attn_w[:, 768 * j:768 * j + 128]
    kc = c_attn_w[:, 768 * j + 512:768 * j + 640]
    q = (x[:32] @ qc + c_attn_b[768 * j:768 * j + 128])[None, None]
    k = (x @ kc + c_attn_b[768 * j + 512:768 * j + 640])[None, None]
    q = _rope_np(q, rope_cos[:32], rope_sin[:32])[0, 0]
    k = _rope_np(k, rope_cos[:256], rope_sin[:256])[0, 0]
    s_ = (q @ k.T) * SCALE
    m = attention_mask[0, 0, :32, :256]
    pos = np.abs(s_).std() * 8.0 + max(0.0, float(m.max()))
    return pos


def kernel(hidden_states, attention_mask, rope_cos, rope_sin,
           c_attn_w, c_attn_b, c_proj_w, c_proj_b):
    global LAST_EXEC_NS, LAST_RESULTS
    hidden_states = np.asarray(hidden_states, dtype=np.float32)
    attention_mask = np.asarray(attention_mask, dtype=np.float32)
    rope_cos = np.asarray(rope_cos, dtype=np.float32)
    rope_sin = np.asarray(rope_sin, dtype=np.float32)
    c_attn_w = np.asarray(c_attn_w, dtype=np.float32)
    c_attn_b = np.asarray(c_attn_b, dtype=np.float32)
    c_proj_w = np.asarray(c_proj_w, dtype=np.float32)
    c_proj_b = np.asarray(c_proj_b, dtype=np.float32)

    if _score_scale_probe(hidden_states, attention_mask, rope_cos,
                          rope_sin, c_attn_w, c_attn_b) > 75.0:
        # scores would overflow fp32 exp without per-row max subtraction;
        # use the exact (slow) host path rather than returning garbage
        LAST_EXEC_NS = None
        return _kernel_numpy(hidden_states, attention_mask, rope_cos,
                             rope_sin, c_attn_w, c_attn_b, c_proj_w,
                             c_proj_b)

    em_cache = [_emask(attention_mask, b) for b in range(B)]
    pattern = _classify(em_cache)
    # safety: every (qc, qs-subtile) needs at least one contributing ks
    # block, else softmax Z would be empty -> fall back to dense pattern
    degenerate = any(
        not any(pattern[qc][kt][j] for kt in range(ST))
        for qc in range(NQC) for j in range(NT))
    if degenerate:
        pattern = tuple(
            tuple(tuple(2 for _ in range(NT)) for _ in range(ST))
            for _ in range(NQC))

    if pattern not in _CACHE:
        _CACHE[pattern] = _build_nc(pattern)
    nc = _CACHE[pattern]
    in_maps = []
    for core in range(8):
        b, g = divmod(core, 4)
        in_maps.append(_prep_core(b, g, hidden_states, attention_mask, em_cache,
                                  rope_cos, rope_sin, c_attn_w, c_attn_b,
                                  c_proj_w, c_proj_b))

    trace = bool(int(os.environ.get("BASS_KERNEL_TRACE", "0")))
    res = run_bass_kernel_spmd(nc, in_maps, list(range(8)), trace=trace)
    LAST_EXEC_NS = res.exec_time_ns
    LAST_RESULTS = res

    out = np.zeros((B, S, H), dtype=np.float32)
    for core in range(8):
        b = core // 4
        out[b] += res.results[core]["out"]
    out += c_proj_b[None, None, :]
    return out
